# revision 1
# baseline (speedup 1.0000x reference)
"""Trainium2 Bass kernel for nn_AttentionBlock (sparse_attention).

Reference computation (N=8192, D=256):
    q = l2norm(x @ Wq.T + bq); k = l2norm(x @ Wk.T + bk); v = x @ Wv.T + bv
    w = relu(q @ k.T); w[diag] = 0; w /= max(rowsum(w), eps)
    out = w @ v + x

Algebraic restructuring used here (all exact up to eps-clamp corner cases
that are probability-zero for random data):
  * relu is positively homogeneous and rows are renormalized by their sum,
    so the q-normalization scale (1/|q_r|) cancels: skip it entirely.
  * The k-normalization column scale cs_j = 1/max(|k_j|, eps) commutes
    through relu: relu(q_r . k_j * cs_j) = cs_j * relu(q_r . k_j).  Fold it
    into v by scaling v rows, and carry cs itself in an extra column to
    recover the row sums (flash-attention style ones-trick).
  * The zeroed diagonal is handled by SUBTRACTING the self term
    m_r = relu(q_r . k_r) * cs_r from both the accumulated numerator
    (m_r * v_r) and the row sum. This keeps the device program identical
    across all 8 cores (pure SPMD; no per-core control flow).

  * Since normalized rows sum to 1, w @ (v + 1*bv) = w @ v + bv: the v
    bias is folded into the host-side residual (xr + bv), removing all
    bias matmul chunks for v.

Sharding: rows of x across 8 cores (SPMD, identical program; per-core
data = x rows slab + replicated x^T/weights). Each core computes its
[1024, 8192] attention slab in [j, r] layout (scores^T), so the relu'd
slab feeds the second matmul as the stationary operand with no
transposes anywhere. The colscale multiply rides the PSUM->SBUF relu
copy (ACT Relu(scale=cs) / DVE (max 0)*cs), keeping the colscale chain
off the critical path.

All matmul operands are bf16 (1 cycle/row on the PE, vs 4 for fp32); PSUM
accumulation and the normalization/epilogue arithmetic stay fp32. The
attention correction is only ~1% of the output magnitude (out ~= x + small
weighted mean of v), so bf16 weight noise lands ~1e-4 relative error.

Pipeline: x^T streams through SBUF in 1024-column chunks producing
k^T/v/colscale; score blocks for the first row-block interleave between
chunks so the PE never drains; PSUM is split 6 rotating work banks + 2
attention accumulators (row-block width 256). Cost-model makespan
~160us/core vs 144us PE busy.
"""

import numpy as np

import concourse.bass as bass
import concourse.bacc as bacc
import concourse.mybir as mybir
from concourse import tile
from concourse.bass_utils import run_bass_kernel_spmd

F32 = mybir.dt.float32
BF16 = mybir.dt.bfloat16
AF = mybir.ActivationFunctionType

M = 8       # cores
N = 8192    # tokens
D = 256     # feature dim

TRACE = False
LAST = None
_CACHE = {}


def build(n=N, r=N // M):
    """Build the single-core SPMD program (phase-fused pipeline)."""
    NJ = n // 128            # 128-wide j blocks
    NCH = n // 1024          # xT streaming chunks
    RT = r // 128            # 128-row subtiles of this core's rows
    RW = min(256, r)         # scores moving width (r columns per block)
    NRB = r // RW            # row blocks
    SS = RW // 128           # 128-row subtiles per row block
    QH = (r + 511) // 512    # q/kself projection column halves
    JPC = 8                  # j blocks per chunk

    nc = bacc.Bacc(None)
    xT_d = nc.declare_dram_parameter("xT", [D, n], BF16, isOutput=False)
    xrT_d = nc.declare_dram_parameter("xrT", [D, r], BF16, isOutput=False)
    xr_d = nc.declare_dram_parameter("xr", [r, D], F32, isOutput=False)
    wq_d = nc.declare_dram_parameter("wqT", [D, D], BF16, isOutput=False)
    wk_d = nc.declare_dram_parameter("wkT", [D, D], BF16, isOutput=False)
    wv_d = nc.declare_dram_parameter("wvT", [D, D], BF16, isOutput=False)
    bq_d = nc.declare_dram_parameter("bq", [128, 2], F32, isOutput=False)
    bk_d = nc.declare_dram_parameter("bk", [128, 2], F32, isOutput=False)
    out_d = nc.declare_dram_parameter("out", [r, D], F32, isOutput=True)

    with tile.TileContext(nc, pool_alloc_mode="queue") as tc:
        with tc.tile_pool(name="pers", bufs=1) as pers, \
             tc.tile_pool(name="p1", bufs=3) as p1, \
             tc.tile_pool(name="p1s", bufs=2) as p1s, \
             tc.tile_pool(name="wtp", bufs=6) as wtp, \
             tc.tile_pool(name="ep", bufs=2) as ep, \
             tc.tile_pool(name="otp", bufs=2) as otp, \
             tc.tile_pool(name="wps", bufs=6, space="PSUM") as wps, \
             tc.tile_pool(name="avp", bufs=1, space="PSUM") as avp:
            # ---- persistent SBUF state ----
            kT = [pers.tile([128, n], BF16, name=f"kT{i}", tag=f"kT{i}") for i in range(2)]
            qT = [pers.tile([128, r], BF16, name=f"qT{i}", tag=f"qT{i}") for i in range(2)]
            vaug = [pers.tile([128, D + 1], BF16, name=f"va{j}", tag=f"va{j}") for j in range(NJ)]
            vself = [pers.tile([128, D], F32, name=f"vs{t}", tag=f"vs{t}") for t in range(RT)]
            xrt = [pers.tile([128, D], F32, name=f"xrs{t}", tag=f"xrs{t}") for t in range(RT)]
            cs = pers.tile([128, NJ], F32, name="cs", tag="cs")
            msb = pers.tile([128, RT], F32, name="msb", tag="msb")
            wqt = [pers.tile([128, D], BF16, name=f"wqt{i}", tag=f"wqt{i}") for i in range(2)]
            wkt = [pers.tile([128, D], BF16, name=f"wkt{i}", tag=f"wkt{i}") for i in range(2)]
            wvt = [pers.tile([128, D], BF16, name=f"wvt{i}", tag=f"wvt{i}") for i in range(2)]
            bq = pers.tile([128, 2], F32, name="bq", tag="bq")
            bk = pers.tile([128, 2], F32, name="bk", tag="bk")
            ones_r = pers.tile([128, 1], BF16, name="ones_r", tag="ones_r")
            xrT = [pers.tile([128, r], BF16, name=f"xrT{i}", tag=f"xrT{i}") for i in range(2)]
            ksf = [pers.tile([128, r], BF16, name=f"ksf{i}", tag=f"ksf{i}") for i in range(2)]

            for i in range(2):
                nc.sync.dma_start(wkt[i][:], wk_d[i * 128:(i + 1) * 128, :])
                nc.gpsimd.dma_start(wvt[i][:], wv_d[i * 128:(i + 1) * 128, :])
                nc.gpsimd.dma_start(wqt[i][:], wq_d[i * 128:(i + 1) * 128, :])
                nc.gpsimd.dma_start(xrT[i][:], xrT_d[i * 128:(i + 1) * 128, :])
            nc.gpsimd.dma_start(bk[:], bk_d[:])
            nc.gpsimd.dma_start(bq[:], bq_d[:])
            for t in range(RT):
                nc.gpsimd.dma_start(xrt[t][:], xr_d[t * 128:(t + 1) * 128, :])
            nc.vector.memset(ones_r[:], 1.0)
            for jb in range(NJ):
                nc.vector.memset(vaug[jb][:, D:D + 1], 1.0)

            def emit_chunk(ch):
                """Stream one 1024-col slab of xT; produce kT, vaug, cs for it."""
                xt = [p1.tile([128, 1024], BF16, name=f"xt{i}", tag=f"xt{i}") for i in range(2)]
                csl = slice(ch * 1024, (ch + 1) * 1024)
                nc.sync.dma_start(xt[0][:], xT_d[0:128, csl])
                nc.sync.dma_start(xt[1][:], xT_d[128:256, csl])
                ksq = [p1s.tile([128, 1024], BF16, name=f"ksq{i}", tag=f"ksq{i}") for i in range(2)]
                for jh in range(2):
                    nsl = slice(ch * 1024 + jh * 512, ch * 1024 + jh * 512 + 512)
                    lsl = slice(jh * 512, jh * 512 + 512)
                    for db in range(2):
                        dsl = slice(db * 128, (db + 1) * 128)
                        ps = wps.tile([128, 512], F32, name="kprj", tag="w")
                        nc.tensor.matmul(ps[:], wkt[0][:, dsl], xt[0][:, lsl], start=True, stop=False)
                        nc.tensor.matmul(ps[:], wkt[1][:, dsl], xt[1][:, lsl], start=False, stop=True)
                        nc.vector.tensor_scalar_add(kT[db][:, nsl], ps[:], bk[:, db:db + 1])
                        nc.scalar.activation(ksq[db][:, lsl], ps[:], AF.Square, bias=bk[:, db:db + 1])
                for t in range(JPC):
                    jb = ch * JPC + t
                    tsl = slice(t * 128, (t + 1) * 128)
                    ps = wps.tile([128, D], F32, name="vprj", tag="w")
                    nc.tensor.matmul(ps[:], xt[0][:, tsl], wvt[0][:], start=True, stop=False)
                    nc.tensor.matmul(ps[:], xt[1][:, tsl], wvt[1][:], start=False, stop=True)
                    if t % 2 == 0:
                        nc.scalar.activation(vaug[jb][:, 0:D], ps[:], AF.Copy)
                    else:
                        nc.vector.tensor_copy(vaug[jb][:, 0:D], ps[:])
                cst = p1s.tile([128, JPC], F32, name="cst", tag="cst")
                crd = wps.tile([128, JPC], F32, name="crd", tag="w")
                for t in range(JPC):
                    tsl = slice(t * 128, (t + 1) * 128)
                    nc.tensor.matmul(crd[:, t:t + 1], ksq[0][:, tsl], ones_r[:], start=True, stop=False)
                    nc.tensor.matmul(crd[:, t:t + 1], ksq[1][:, tsl], ones_r[:], start=False, stop=True)
                nc.vector.tensor_scalar_add(cst[:], crd[:], 1e-24)
                css = p1s.tile([128, JPC], F32, name="css", tag="css")
                nc.scalar.sqrt(css[:], cst[:])
                nc.vector.reciprocal(cs[:, ch * JPC:(ch + 1) * JPC], css[:])

            def emit_1bproj():
                """q / k_self / v_self projections for this core's rows."""
                for rh in range(QH):
                    w = min(512, r - rh * 512)
                    sl = slice(rh * 512, rh * 512 + w)
                    for db in range(2):
                        dsl = slice(db * 128, (db + 1) * 128)
                        ps = wps.tile([128, 512], F32, name="qps", tag="w")
                        nc.tensor.matmul(ps[:, :w], wqt[0][:, dsl], xrT[0][:, sl], start=True, stop=False)
                        nc.tensor.matmul(ps[:, :w], wqt[1][:, dsl], xrT[1][:, sl], start=False, stop=True)
                        nc.scalar.activation(qT[db][:, sl], ps[:, :w], AF.Identity, bias=bq[:, db:db + 1])
                        ps2 = wps.tile([128, 512], F32, name="kps", tag="w")
                        nc.tensor.matmul(ps2[:, :w], wkt[0][:, dsl], xrT[0][:, sl], start=True, stop=False)
                        nc.tensor.matmul(ps2[:, :w], wkt[1][:, dsl], xrT[1][:, sl], start=False, stop=True)
                        nc.vector.tensor_scalar_add(ksf[db][:, sl], ps2[:, :w], bk[:, db:db + 1])
                for t in range(RT):
                    tsl = slice(t * 128, (t + 1) * 128)
                    ps = wps.tile([128, D], F32, name="vps", tag="w")
                    nc.tensor.matmul(ps[:], xrT[0][:, tsl], wvt[0][:], start=True, stop=False)
                    nc.tensor.matmul(ps[:], xrT[1][:, tsl], wvt[1][:], start=False, stop=True)
                    nc.scalar.activation(vself[t][:], ps[:], AF.Copy)

            def emit_selfterm():
                """m = relu(diag(q.k_self)) / |k_self| for the diagonal subtraction."""
                qk = [p1s.tile([128, r], BF16, name=f"qk{i}", tag=f"qk{i}") for i in range(2)]
                qs = [p1s.tile([128, r], BF16, name=f"qs{i}", tag=f"qs{i}") for i in range(2)]
                nc.scalar.square(qs[0][:], ksf[0][:])
                nc.scalar.square(qs[1][:], ksf[1][:])
                nc.vector.tensor_mul(qk[0][:], qT[0][:], ksf[0][:])
                nc.vector.tensor_mul(qk[1][:], qT[1][:], ksf[1][:])
                nc.vector.tensor_add(qk[0][:], qk[0][:], qk[1][:])
                sdt = p1s.tile([128, RT], F32, name="sdt", tag="sdt")
                sdp = wps.tile([128, RT], F32, name="sdp", tag="w")
                ksp = wps.tile([128, RT], F32, name="ksp", tag="w")
                for t in range(RT):
                    tsl = slice(t * 128, (t + 1) * 128)
                    nc.tensor.matmul(ksp[:, t:t + 1], qs[0][:, tsl], ones_r[:], start=True, stop=False)
                    nc.tensor.matmul(ksp[:, t:t + 1], qs[1][:, tsl], ones_r[:], start=False, stop=True)
                    nc.tensor.matmul(sdp[:, t:t + 1], qk[0][:, tsl], ones_r[:], start=True, stop=True)
                nc.vector.tensor_copy(sdt[:], sdp[:])
                kst = p1s.tile([128, RT], F32, name="kst", tag="kst")
                nc.vector.tensor_scalar_add(kst[:], ksp[:], 1e-24)
                kss = p1s.tile([128, RT], F32, name="kss", tag="kss")
                nc.scalar.sqrt(kss[:], kst[:])
                inv = p1s.tile([128, RT], F32, name="inv", tag="inv")
                nc.vector.reciprocal(inv[:], kss[:])
                nc.vector.tensor_scalar_max(sdt[:], sdt[:], 0.0)
                nc.vector.tensor_mul(msb[:], sdt[:], inv[:])

            def emit_jbs(rb, av, jb_lo, jb_hi):
                rsl = slice(rb * RW, rb * RW + RW)
                for jb in range(jb_lo, jb_hi):
                    jsl = slice(jb * 128, (jb + 1) * 128)
                    sc = wps.tile([128, RW], F32, name="sc", tag="w")
                    nc.tensor.matmul(sc[:], kT[0][:, jsl], qT[0][:, rsl], start=True, stop=False)
                    nc.tensor.matmul(sc[:], kT[1][:, jsl], qT[1][:, rsl], start=False, stop=True)
                    wt = wtp.tile([128, RW], BF16, name="wt", tag="wt")
                    if jb % 2 == 0 or jb % 16 == 1:
                        nc.vector.tensor_scalar(out=wt[:], in0=sc[:], scalar1=0.0,
                                                scalar2=cs[:, jb:jb + 1],
                                                op0=mybir.AluOpType.max,
                                                op1=mybir.AluOpType.mult)
                    else:
                        nc.scalar.activation(wt[:], sc[:], AF.Relu, scale=cs[:, jb:jb + 1])
                    for s in range(SS):
                        nc.tensor.matmul(av[s][:], wt[:, s * 128:(s + 1) * 128], vaug[jb][:],
                                         start=(jb == 0), stop=(jb == NJ - 1))

            def emit_epilogue(rb, av):
                for s in range(SS):
                    t = rb * SS + s
                    tmp = ep.tile([128, D], F32, name="tmp", tag="tmp")
                    nc.scalar.activation(tmp[:], vself[t][:], AF.Copy, scale=msb[:, t:t + 1])
                    num = ep.tile([128, D], F32, name="num", tag="num")
                    nc.vector.tensor_sub(num[:], av[s][:, 0:D], tmp[:])
                    den = ep.tile([128, 1], F32, name="den", tag="den")
                    nc.vector.tensor_scalar_sub(den[:], av[s][:, D:D + 1], msb[:, t:t + 1])
                    nc.vector.tensor_scalar_add(den[:], den[:], 1e-9)
                    rec = ep.tile([128, 1], F32, name="rec", tag="rec")
                    nc.vector.reciprocal(rec[:], den[:])
                    ot = otp.tile([128, D], F32, name="ot", tag="ot")
                    nc.scalar.activation(ot[:], num[:], AF.Copy, scale=rec[:])
                    nc.vector.tensor_add(ot[:], ot[:], xrt[t][:])
                    nc.sync.dma_start(out_d[t * 128:(t + 1) * 128, :], ot[:])

            # ---- fused pipeline ----
            emit_chunk(0)
            emit_1bproj()
            av0 = [avp.tile([128, D + 1], F32, name=f"av{s}", tag=f"av{s}") for s in range(SS)]
            for ch in range(1, NCH):
                emit_chunk(ch)
                emit_jbs(0, av0, (ch - 1) * JPC, ch * JPC)
            emit_jbs(0, av0, (NCH - 1) * JPC, NJ)
            emit_selfterm()
            emit_epilogue(0, av0)
            for rb in range(1, NRB):
                av = [avp.tile([128, D + 1], F32, name=f"av{s}", tag=f"av{s}") for s in range(SS)]
                emit_jbs(rb, av, 0, NJ)
                emit_epilogue(rb, av)
    nc.compile()
    return nc


def _get_nc(n=N, r=N // M):
    key = (n, r)
    if key not in _CACHE:
        _CACHE[key] = build(n, r)
    return _CACHE[key]


def kernel(x, Wq, bq, Wk, bk, Wv, bv):
    global LAST
    bf16 = mybir.dt.np(BF16)
    x = np.ascontiguousarray(np.asarray(x, np.float32))
    n = x.shape[0]
    r = n // M
    xb = x.astype(bf16)
    xT = np.ascontiguousarray(xb.T)
    wqT = np.ascontiguousarray(np.asarray(Wq, np.float32).T.astype(bf16))
    wkT = np.ascontiguousarray(np.asarray(Wk, np.float32).T.astype(bf16))
    wvT = np.ascontiguousarray(np.asarray(Wv, np.float32).T.astype(bf16))
    xplus = x + np.asarray(bv, np.float32)[None, :]
    bq2 = np.ascontiguousarray(np.asarray(bq, np.float32).reshape(2, 128).T)
    bk2 = np.ascontiguousarray(np.asarray(bk, np.float32).reshape(2, 128).T)
    in_maps = []
    for c in range(M):
        rows = slice(c * r, (c + 1) * r)
        in_maps.append({
            "xT": xT,
            "xrT": np.ascontiguousarray(xb[rows].T),
            "xr": np.ascontiguousarray(xplus[rows]),
            "wqT": wqT, "wkT": wkT, "wvT": wvT,
            "bq": bq2, "bk": bk2,
        })
    res = run_bass_kernel_spmd(_get_nc(n, r), in_maps, core_ids=list(range(M)), trace=TRACE)
    LAST = res
    return np.concatenate([res.results[c]["out"] for c in range(M)], axis=0)



# revision 23
# speedup vs baseline: 2.0086x; 2.0086x over previous
"""Trainium2 Bass kernel for nn_AttentionBlock (sparse_attention).

Reference computation (N=8192, D=256):
    q = l2norm(x @ Wq.T + bq); k = l2norm(x @ Wk.T + bk); v = x @ Wv.T + bv
    w = relu(q @ k.T); w[diag] = 0; w /= max(rowsum(w), eps)
    out = w @ v + x

Algebraic restructuring (same as the bf16 baseline):
  * relu is positively homogeneous and rows are renormalized by their sum,
    so the q-normalization scale cancels: skip it entirely.
  * The k-normalization column scale cs_j = 1/|k_j| commutes through relu.
    It is folded into v (v rows scaled by cs_j at the v evacuation) and
    carried as an fp8 copy of cs for the denominator row sums
    (flash-attention ones-trick, with cs8 as the moving operand).
  * The zeroed diagonal is handled by subtracting a separately computed self
    term m_r = relu(q_r . k_r)/|k_r| from numerator (m_r * v_r) and row sum.
  * v bias folded into the host-side residual (xr + bv).

Speed: all big matmuls run in fp8 (e4m3) with MatmulPerfMode.DoubleRow:
one matmul contracts 2x128 partitions at 0.5 cycles/row, 4x fewer PE
cycles than bf16 pairing for the same D=256 contraction.  fp8 operands
live in "pair layout" [128, 2, n]: partition p, pair i holds contraction
element i*128+p.  Scores for 4 j-blocks accumulate in one 2-bank PSUM
tile [128, 1024] and leave through a single pure-relu evacuation into
wt4 [128, 4, 256] fp8, which directly exposes the DoubleRow stationary
pairs for the w @ v matmul.

The PSUM evacuations (relu on scores, bias-adds on k/q, cs-scaled copies
of v) are the bottleneck; only DVE and ACT can read PSUM, so they split
that work while GPSIMD (Pool) takes SBUF-side work (k^2 for the column
norms, q*k self-term products, epilogue arithmetic).  The kernel runs in
two sequential phases so each gets the PSUM banks it needs: phase A
(projections; 6 rotating work banks) then phase B (scores + w@v; 3
double-bank score tiles + 2 accumulator banks).

Numerics (numpy emulation of the full fp8 pipeline): rel err ~2.7e-3
vs the fp32 reference, comfortably under the 2e-2 gate.
"""

import numpy as np

import concourse.bass as bass
import concourse.bacc as bacc
import concourse.mybir as mybir
from concourse import tile
from concourse.bass_utils import run_bass_kernel_spmd

F32 = mybir.dt.float32
BF16 = mybir.dt.bfloat16
F8 = mybir.dt.float8e4
AF = mybir.ActivationFunctionType
PM = mybir.MatmulPerfMode
ALU = mybir.AluOpType

M = 8       # cores
N = 8192    # tokens
D = 256     # feature dim

TRACE = False
LAST = None
_CACHE = {}

# engine split knobs (tuned against the scheduler makespan)
SC_PAT = "ADADADADADADADADA" + "DADADADADADADAD"  # score evacs: A=ACT, D=DVE
KV_PAT = "AD"                  # k/q/ksf/vself/v psum evacs
VS_PAT = "P"                   # in-place cs scaling of v (SBUF)
KSQ_PAT = "PPPD"               # ksq: P=Pool, D=DVE


def build(n=N, r=N // M):
    NJ = n // 128            # 64  j blocks
    CH = n // 1024           # 8   xT streaming chunks (1024 j each)
    RT = r // 128            # 8   128-row tiles of this core's rows
    RW = 256                 # row-block width (r cols per score group)
    NRB = r // RW            # 4   row blocks
    SS = RW // 128           # 2   128-row subtiles per row block
    GJB = 4                  # j blocks per score psum group
    NG = NJ // GJB           # 16  score groups per row block

    nc = bacc.Bacc(None)
    xTp_d = nc.declare_dram_parameter("xTp", [128, 2, n], F8, isOutput=False)
    xrTp_d = nc.declare_dram_parameter("xrTp", [128, 2, r], F8, isOutput=False)
    xr_d = nc.declare_dram_parameter("xr", [r, D], F32, isOutput=False)
    wq_d = nc.declare_dram_parameter("wqTp", [128, 2, D], F8, isOutput=False)
    wk_d = nc.declare_dram_parameter("wkTp", [128, 2, D], F8, isOutput=False)
    wv_d = nc.declare_dram_parameter("wvTp", [128, 2, D], F8, isOutput=False)
    bq_d = nc.declare_dram_parameter("bq2", [128, 2], F32, isOutput=False)
    bk_d = nc.declare_dram_parameter("bk2", [128, 2], F32, isOutput=False)
    out_d = nc.declare_dram_parameter("out", [r, D], F32, isOutput=True)

    def mk_cycle(pat, m):
        state = {"i": 0}
        def nxt():
            e = m[pat[state["i"] % len(pat)]]
            state["i"] += 1
            return e
        return nxt

    with tile.TileContext(nc, pool_alloc_mode="queue") as tc:
        with tc.tile_pool(name="pers", bufs=1) as pers, \
             tc.tile_pool(name="xtp", bufs=2) as xtp, \
             tc.tile_pool(name="ksqp", bufs=2) as ksqp, \
             tc.tile_pool(name="wtp", bufs=4) as wtp, \
             tc.tile_pool(name="ep", bufs=2) as ep, \
             tc.tile_pool(name="otp", bufs=2) as otp:
            emap = {"A": nc.scalar, "D": nc.vector, "P": nc.gpsimd}
            sc_eng = mk_cycle(SC_PAT, emap)
            kv_eng = mk_cycle(KV_PAT, emap)
            vs_eng = mk_cycle(VS_PAT, emap)
            ksq_eng = mk_cycle(KSQ_PAT, emap)

            # ---- persistent SBUF state ----
            kTp = pers.tile([128, 2, n], F8, name="kTp", tag="kTp")
            qTp = pers.tile([128, 2, r], F8, name="qTp", tag="qTp")
            vp = [pers.tile([128, 2, D], F8, name=f"vp{jj}", tag=f"vp{jj}")
                  for jj in range(NJ // 2)]
            cs = pers.tile([128, NJ], F32, name="cs", tag="cs")
            cs8 = pers.tile([128, NJ, 1], F8, name="cs8", tag="cs8")
            wqt = pers.tile([128, 2, D], F8, name="wqt", tag="wqt")
            wkt = pers.tile([128, 2, D], F8, name="wkt", tag="wkt")
            wvt = pers.tile([128, 2, D], F8, name="wvt", tag="wvt")
            bq2 = pers.tile([128, 2], F32, name="bq2", tag="bq2")
            bk2 = pers.tile([128, 2], F32, name="bk2", tag="bk2")
            ones8 = pers.tile([128, 2, 1], F8, name="ones8", tag="ones8")
            ksf = pers.tile([128, 2, r], BF16, name="ksf", tag="ksf")
            qk8 = pers.tile([128, 2, r], F8, name="qk8", tag="qk8")
            qs8 = pers.tile([128, 2, r], F8, name="qs8", tag="qs8")
            vself = [pers.tile([128, 2, D], F32, name=f"vs{t}", tag=f"vs{t}")
                     for t in range(RT // 2)]
            xrt = [pers.tile([128, D], F32, name=f"xrs{t}", tag=f"xrs{t}")
                   for t in range(RT)]
            msb = pers.tile([128, RT], F32, name="msb", tag="msb")
            xrTp = pers.tile([128, 2, r], F8, name="xrTp", tag="xrTp")

            nc.gpsimd.dma_start(wkt[:], wk_d[:])
            nc.gpsimd.dma_start(wvt[:], wv_d[:])
            nc.gpsimd.dma_start(wqt[:], wq_d[:])
            nc.gpsimd.dma_start(xrTp[:], xrTp_d[:])
            nc.gpsimd.dma_start(bk2[:], bk_d[:])
            nc.gpsimd.dma_start(bq2[:], bq_d[:])
            nc.vector.memset(ones8[:], 1.0)
            epsb = pers.tile([128, 1], F32, name="epsb", tag="epsb")
            nc.vector.memset(epsb[:], 1e-24)

            def evac(eng, out_ap, in_ap, bias=None, scale=None, relu=False):
                """PSUM -> SBUF evacuation on ACT or DVE."""
                if eng is nc.scalar:
                    func = AF.Relu if relu else (AF.Identity if bias is not None
                                                 else AF.Copy)
                    kw = {}
                    if bias is not None:
                        kw["bias"] = bias
                    if scale is not None:
                        kw["scale"] = scale
                    nc.scalar.activation(out_ap, in_ap, func, **kw)
                else:
                    if relu:
                        if scale is not None:
                            eng.tensor_scalar(out=out_ap, in0=in_ap,
                                              scalar1=0.0, scalar2=scale,
                                              op0=ALU.max, op1=ALU.mult)
                        else:
                            eng.tensor_scalar(out=out_ap, in0=in_ap,
                                              scalar1=0.0, scalar2=None,
                                              op0=ALU.max)
                    elif bias is not None:
                        eng.tensor_scalar_add(out_ap, in_ap, bias)
                    elif scale is not None:
                        eng.tensor_scalar_mul(out_ap, in_ap, scale)
                    else:
                        eng.tensor_copy(out_ap, in_ap)

            xts = {}

            def fetch(ch):
                if ch >= CH or ch in xts:
                    return
                xt = xtp.tile([128, 2, 1024], F8, name="xt", tag="xt")
                nc.sync.dma_start(xt[:], xTp_d[:, :, ch * 1024:(ch + 1) * 1024])
                xts[ch] = xt

            def emit_chunk(ch, wp):
                """Stream one 1024-col slab of xTp; produce kTp, vp, cs."""
                xt = xts.pop(ch)
                fetch(ch + 1)
                crd = wp.tile([128, 8], F32, name="crd", tag="crd", bufs=2)
                for jh in range(2):
                    for db in range(2):
                        kps = wp.tile([128, 512], F32, name="kps", tag="w")
                        for m in range(2):
                            lsl = slice(jh * 512 + m * 256,
                                        jh * 512 + (m + 1) * 256)
                            nc.tensor.matmul(kps[:, m * 256:(m + 1) * 256],
                                             wkt[:, :, db * 128:(db + 1) * 128],
                                             xt[:, :, lsl],
                                             start=True, stop=True,
                                             perf_mode=PM.DoubleRow)
                        gsl = slice(ch * 1024 + jh * 512,
                                    ch * 1024 + (jh + 1) * 512)
                        evac(kv_eng(), kTp[:, db, gsl], kps[:],
                             bias=bk2[:, db:db + 1])
                    # ksq from the fp8 kT slab (SBUF), column norms via PE
                    gsl = slice(ch * 1024 + jh * 512, ch * 1024 + (jh + 1) * 512)
                    ksq = ksqp.tile([128, 2, 512], F8, name="ksq", tag="ksq")
                    ke = ksq_eng()
                    ke.tensor_tensor(out=ksq[:], in0=kTp[:, :, gsl],
                                     in1=kTp[:, :, gsl], op=ALU.mult)
                    for t in range(4):
                        col = jh * 4 + t
                        nc.tensor.matmul(crd[:, col:col + 1],
                                         ksq[:, :, t * 128:(t + 1) * 128],
                                         ones8[:],
                                         start=True, stop=True,
                                         perf_mode=PM.DoubleRow)
                    # cs for this half chunk (shortens the v-evac chain)
                    hs = slice(ch * 8 + jh * 4, ch * 8 + (jh + 1) * 4)
                    cl = slice(jh * 4, (jh + 1) * 4)
                    csn = ep.tile([128, 4], F32, name="csn", tag="csn")
                    nc.scalar.activation(csn[:], crd[:, cl], AF.Sqrt,
                                         bias=epsb[:])
                    nc.vector.reciprocal(cs[:, hs], csn[:])
                    nc.gpsimd.tensor_copy(cs8[:, hs, 0:1], cs[:, hs])
                # v projection: [j, d] psums, 2 j-blocks per bank;
                # pure evac, then per-jb cs scaling in SBUF (Pool-friendly)
                for pj in range(4):
                    jj = ch * 4 + pj
                    vps = wp.tile([128, 512], F32, name="vps", tag="w")
                    for i in range(2):
                        lsl = slice((pj * 2 + i) * 128, (pj * 2 + i + 1) * 128)
                        nc.tensor.matmul(vps[:, i * 256:(i + 1) * 256],
                                         xt[:, :, lsl], wvt[:],
                                         start=True, stop=True,
                                         perf_mode=PM.DoubleRow)
                    evac(kv_eng(), vp[jj][:], vps[:])
                    for i in range(2):
                        jb = 2 * jj + i
                        se = vs_eng()
                        if se is nc.scalar:
                            nc.scalar.activation(vp[jj][:, i, :],
                                                 vp[jj][:, i, :], AF.Copy,
                                                 scale=cs[:, jb:jb + 1])
                        else:
                            se.tensor_scalar(out=vp[jj][:, i, :],
                                             in0=vp[jj][:, i, :],
                                             scalar1=cs[:, jb:jb + 1],
                                             scalar2=None, op0=ALU.mult)

            def emit_rproj(wp):
                """q / k_self / v_self projections for this core's rows."""
                for db in range(2):
                    for rh in range(r // 512):
                        rsl = slice(rh * 512, (rh + 1) * 512)
                        qps = wp.tile([128, 512], F32, name="qps", tag="w")
                        for m in range(2):
                            msl = slice(rh * 512 + m * 256,
                                        rh * 512 + (m + 1) * 256)
                            nc.tensor.matmul(qps[:, m * 256:(m + 1) * 256],
                                             wqt[:, :, db * 128:(db + 1) * 128],
                                             xrTp[:, :, msl],
                                             start=True, stop=True,
                                             perf_mode=PM.DoubleRow)
                        evac(kv_eng(), qTp[:, db, rsl], qps[:],
                             bias=bq2[:, db:db + 1])
                        kps = wp.tile([128, 512], F32, name="ksps", tag="w")
                        for m in range(2):
                            msl = slice(rh * 512 + m * 256,
                                        rh * 512 + (m + 1) * 256)
                            nc.tensor.matmul(kps[:, m * 256:(m + 1) * 256],
                                             wkt[:, :, db * 128:(db + 1) * 128],
                                             xrTp[:, :, msl],
                                             start=True, stop=True,
                                             perf_mode=PM.DoubleRow)
                        evac(kv_eng(), ksf[:, db, rsl], kps[:],
                             bias=bk2[:, db:db + 1])
                for tt in range(RT // 2):
                    vsp = wp.tile([128, 512], F32, name="vsp", tag="w")
                    for i in range(2):
                        t = 2 * tt + i
                        nc.tensor.matmul(vsp[:, i * 256:(i + 1) * 256],
                                         xrTp[:, :, t * 128:(t + 1) * 128],
                                         wvt[:],
                                         start=True, stop=True,
                                         perf_mode=PM.DoubleRow)
                    evac(kv_eng(), vself[tt][:], vsp[:])

            def emit_selfterm(wp):
                """m = relu(diag(q.k_self)) / |k_self| for diagonal removal."""
                for h in range(2):
                    hsl = slice(h * 512, (h + 1) * 512)
                    nc.gpsimd.tensor_tensor(out=qk8[:, :, hsl],
                                            in0=qTp[:, :, hsl],
                                            in1=ksf[:, :, hsl], op=ALU.mult)
                    nc.scalar.activation(qs8[:, :, hsl], ksf[:, :, hsl],
                                         AF.Square)
                sdp = wp.tile([128, RT], F32, name="sdp", tag="crd", bufs=2)
                ksp = wp.tile([128, RT], F32, name="ksp", tag="crd", bufs=2)
                for t in range(RT):
                    tsl = slice(t * 128, (t + 1) * 128)
                    nc.tensor.matmul(sdp[:, t:t + 1], qk8[:, :, tsl], ones8[:],
                                     start=True, stop=True,
                                     perf_mode=PM.DoubleRow)
                    nc.tensor.matmul(ksp[:, t:t + 1], qs8[:, :, tsl], ones8[:],
                                     start=True, stop=True,
                                     perf_mode=PM.DoubleRow)
                kst = ep.tile([128, RT], F32, name="kst", tag="kst")
                nc.scalar.activation(kst[:], ksp[:], AF.Sqrt, bias=epsb[:])
                inv = ep.tile([128, RT], F32, name="inv", tag="inv")
                nc.vector.reciprocal(inv[:], kst[:])
                nc.vector.tensor_scalar(out=msb[:], in0=sdp[:],
                                        scalar1=0.0, scalar2=None, op0=ALU.max)
                nc.gpsimd.tensor_tensor(out=msb[:], in0=msb[:], in1=inv[:],
                                        op=ALU.mult)

            def emit_scores(rb, g, sp, avm, avo):
                """4 j-blocks of scores -> relu -> fp8 wt4 -> AV accumulate."""
                rsl = slice(rb * RW, (rb + 1) * RW)
                sc = sp.tile([128, 1024], F32, name="sc", tag="sc")
                for i in range(GJB):
                    jb = g * GJB + i
                    nc.tensor.matmul(sc[:, i * 256:(i + 1) * 256],
                                     kTp[:, :, jb * 128:(jb + 1) * 128],
                                     qTp[:, :, rsl],
                                     start=True, stop=True,
                                     perf_mode=PM.DoubleRow)
                wt4 = wtp.tile([128, GJB, 256], F8, name="wt4", tag="wt4")
                evac(sc_eng(), wt4[:], sc[:], relu=True)
                for pj in range(GJB // 2):
                    jj = g * 2 + pj
                    for s in range(SS):
                        ssl = slice(s * 128, (s + 1) * 128)
                        nc.tensor.matmul(avm[:, s, :],
                                         wt4[:, 2 * pj:2 * pj + 2, ssl],
                                         vp[jj][:],
                                         start=(jj == 0),
                                         stop=(jj == NJ // 2 - 1),
                                         perf_mode=PM.DoubleRow)
                        nc.tensor.matmul(avo[:, s:s + 1],
                                         wt4[:, 2 * pj:2 * pj + 2, ssl],
                                         cs8[:, 2 * jj:2 * jj + 2, :],
                                         start=(jj == 0),
                                         stop=(jj == NJ // 2 - 1),
                                         perf_mode=PM.DoubleRow)

            def emit_epilogue(rb, avm, avo):
                avv = ep.tile([128, SS, 256], F32, name="avv", tag="avv")
                nc.scalar.activation(avv[:], avm[:], AF.Copy)
                dens = ep.tile([128, SS], F32, name="dens", tag="dens")
                nc.vector.tensor_scalar_add(dens[:], avo[:], 1e-9)
                for s in range(SS):
                    t = rb * SS + s
                    tmp = ep.tile([128, D], F32, name="tmp", tag="tmp")
                    nc.gpsimd.tensor_scalar(out=tmp[:],
                                            in0=vself[t // 2][:, t % 2, :],
                                            scalar1=msb[:, t:t + 1],
                                            scalar2=None, op0=ALU.mult)
                    num2 = ep.tile([128, D], F32, name="num2", tag="num2")
                    nc.gpsimd.tensor_tensor(out=num2[:], in0=avv[:, s, :],
                                            in1=tmp[:], op=ALU.subtract)
                    den = ep.tile([128, 1], F32, name="den", tag="den")
                    nc.vector.tensor_scalar(out=den[:], in0=dens[:, s:s + 1],
                                            scalar1=msb[:, t:t + 1],
                                            scalar2=None, op0=ALU.subtract)
                    rec = ep.tile([128, 1], F32, name="rec", tag="rec")
                    nc.vector.reciprocal(rec[:], den[:])
                    ot1 = ep.tile([128, D], F32, name="ot1", tag="ot1")
                    nc.gpsimd.tensor_scalar(out=ot1[:], in0=num2[:],
                                            scalar1=rec[:], scalar2=None,
                                            op0=ALU.mult)
                    ot = otp.tile([128, D], F32, name="ot", tag="ot")
                    nc.gpsimd.tensor_tensor(out=ot[:], in0=ot1[:],
                                            in1=xrt[t][:], op=ALU.add)
                    nc.sync.dma_start(out_d[t * 128:(t + 1) * 128, :], ot[:])

            # ---- phase A: all projections (6 rotating PSUM work banks) ----
            with tc.tile_pool(name="wp", bufs=6, space="PSUM") as wp:
                fetch(0)
                emit_chunk(0, wp)
                emit_rproj(wp)
                emit_selfterm(wp)
                for t in range(RT):
                    nc.gpsimd.dma_start(xrt[t][:],
                                        xr_d[t * 128:(t + 1) * 128, :])
                for ch in range(1, CH):
                    emit_chunk(ch, wp)

            # ---- phase B: scores + w@v (3 x 2-bank scores + 2 accum) ----
            with tc.tile_pool(name="sp", bufs=3, space="PSUM") as sp, \
                 tc.tile_pool(name="avp", bufs=1, space="PSUM") as avp:
                for rb in range(NRB):
                    avm = avp.tile([128, SS, 256], F32, name="avm", tag="avm")
                    avo = avp.tile([128, SS], F32, name="avo", tag="avo")
                    for g in range(NG):
                        emit_scores(rb, g, sp, avm, avo)
                    emit_epilogue(rb, avm, avo)
    nc.compile()
    return nc


def _get_nc(n=N, r=N // M):
    key = (n, r)
    if key not in _CACHE:
        _CACHE[key] = build(n, r)
    return _CACHE[key]


def _pairT(a2d):
    """[n, 256] -> fp8 pair layout [128, 2, n] (transposed)."""
    f8 = mybir.dt.np(F8)
    a = np.asarray(a2d, np.float32).astype(f8)
    n = a.shape[0]
    return np.ascontiguousarray(a.T.reshape(2, 128, n).transpose(1, 0, 2))


def kernel(x, Wq, bq, Wk, bk, Wv, bv):
    global LAST
    x = np.ascontiguousarray(np.asarray(x, np.float32))
    n = x.shape[0]
    r = n // M
    xTp = _pairT(x)
    wqTp = _pairT(np.asarray(Wq, np.float32))   # == Wq.T in pair layout
    wkTp = _pairT(np.asarray(Wk, np.float32))
    wvTp = _pairT(np.asarray(Wv, np.float32))
    xplus = x + np.asarray(bv, np.float32)[None, :]
    bq2 = np.ascontiguousarray(np.asarray(bq, np.float32).reshape(2, 128).T)
    bk2 = np.ascontiguousarray(np.asarray(bk, np.float32).reshape(2, 128).T)
    in_maps = []
    for c in range(M):
        rows = slice(c * r, (c + 1) * r)
        in_maps.append({
            "xTp": xTp,
            "xrTp": _pairT(x[rows]),
            "xr": np.ascontiguousarray(xplus[rows]),
            "wqTp": wqTp, "wkTp": wkTp, "wvTp": wvTp,
            "bq2": bq2, "bk2": bk2,
        })
    res = run_bass_kernel_spmd(_get_nc(n, r), in_maps, core_ids=list(range(M)),
                               trace=TRACE)
    LAST = res
    return np.concatenate([res.results[c]["out"] for c in range(M)], axis=0)


# revision 35
# speedup vs baseline: 2.0120x; 1.0017x over previous
"""Trainium2 Bass kernel for nn_AttentionBlock (sparse_attention).

Reference computation (N=8192, D=256):
    q = l2norm(x @ Wq.T + bq); k = l2norm(x @ Wk.T + bk); v = x @ Wv.T + bv
    w = relu(q @ k.T); w[diag] = 0; w /= max(rowsum(w), eps)
    out = w @ v + x

Algebraic restructuring (same as the bf16 baseline):
  * relu is positively homogeneous and rows are renormalized by their sum,
    so the q-normalization scale cancels: skip it entirely.
  * The k-normalization column scale cs_j = 1/|k_j| commutes through relu.
    It is folded into v (v rows scaled by cs_j at the v evacuation) and
    carried as an fp8 copy of cs for the denominator row sums
    (flash-attention ones-trick, with cs8 as the moving operand).
  * The zeroed diagonal is handled by subtracting a separately computed self
    term m_r = relu(q_r . k_r)/|k_r| from numerator (m_r * v_r) and row sum.
  * v bias folded into the host-side residual (xr + bv).

Speed: all big matmuls run in fp8 (e4m3) with MatmulPerfMode.DoubleRow:
one matmul contracts 2x128 partitions at 0.5 cycles/row, 4x fewer PE
cycles than bf16 pairing for the same D=256 contraction.  fp8 operands
live in "pair layout" [128, 2, n]: partition p, pair i holds contraction
element i*128+p.  Scores for 4 j-blocks accumulate in one 2-bank PSUM
tile [128, 1024] and leave through a single pure-relu evacuation into
wt4 [128, 4, 256] fp8, which directly exposes the DoubleRow stationary
pairs for the w @ v matmul.

The PSUM evacuations (relu on scores, bias-adds on k/q, cs-scaled copies
of v) are the bottleneck; only DVE and ACT can read PSUM, so they split
that work while GPSIMD (Pool) takes SBUF-side work (k^2 for the column
norms, q*k self-term products, epilogue arithmetic).  The kernel runs in
two sequential phases so each gets the PSUM banks it needs: phase A
(projections; 6 rotating work banks) then phase B (scores + w@v; 3
double-bank score tiles + 2 accumulator banks).

Numerics (numpy emulation of the full fp8 pipeline): rel err ~2.7e-3
vs the fp32 reference, comfortably under the 2e-2 gate.
"""

import numpy as np

import concourse.bass as bass
import concourse.bacc as bacc
import concourse.mybir as mybir
from concourse import tile
from concourse.bass_utils import run_bass_kernel_spmd

F32 = mybir.dt.float32
BF16 = mybir.dt.bfloat16
F8 = mybir.dt.float8e4
AF = mybir.ActivationFunctionType
PM = mybir.MatmulPerfMode
ALU = mybir.AluOpType

M = 8       # cores
N = 8192    # tokens
D = 256     # feature dim

TRACE = False
LAST = None
_CACHE = {}

# engine split knobs (tuned against the scheduler makespan)
import os
SC_PAT = os.environ.get("K_SC", "AD")
KV_PAT = os.environ.get("K_KV", "AD")    # k/q/ksf/vself/v psum evacs
VS_PAT = os.environ.get("K_VS", "P")     # in-place cs scaling of v (SBUF)
KSQ_PAT = os.environ.get("K_KSQ", "PPPD")  # ksq: P=Pool, D=DVE
QS_PAT = os.environ.get("K_QS", "A")     # qs squares: A=ACT, P=Pool


def build(n=N, r=N // M):
    NJ = n // 128            # 64  j blocks
    CH = n // 1024           # 8   xT streaming chunks (1024 j each)
    RT = r // 128            # 8   128-row tiles of this core's rows
    RW = 256                 # row-block width (r cols per score group)
    NRB = r // RW            # 4   row blocks
    SS = RW // 128           # 2   128-row subtiles per row block
    GJB = 4                  # j blocks per score psum group
    NG = NJ // GJB           # 16  score groups per row block

    nc = bacc.Bacc(None)
    xTp_d = nc.declare_dram_parameter("xTp", [128, 2, n], F8, isOutput=False)
    xrTp_d = nc.declare_dram_parameter("xrTp", [128, 2, r], F8, isOutput=False)
    xr_d = nc.declare_dram_parameter("xr", [r, D], F32, isOutput=False)
    wq_d = nc.declare_dram_parameter("wqTp", [128, 2, D], F8, isOutput=False)
    wk_d = nc.declare_dram_parameter("wkTp", [128, 2, D], F8, isOutput=False)
    wv_d = nc.declare_dram_parameter("wvTp", [128, 2, D], F8, isOutput=False)
    bq_d = nc.declare_dram_parameter("bq2", [128, 2], F32, isOutput=False)
    bk_d = nc.declare_dram_parameter("bk2", [128, 2], F32, isOutput=False)
    out_d = nc.declare_dram_parameter("out", [r, D], F32, isOutput=True)

    def mk_cycle(pat, m):
        state = {"i": 0}
        def nxt():
            e = m[pat[state["i"] % len(pat)]]
            state["i"] += 1
            return e
        return nxt

    with tile.TileContext(nc, pool_alloc_mode="queue") as tc:
        with tc.tile_pool(name="pers", bufs=1) as pers, \
             tc.tile_pool(name="xtp", bufs=2) as xtp, \
             tc.tile_pool(name="ksqp", bufs=2) as ksqp, \
             tc.tile_pool(name="wtp", bufs=4) as wtp, \
             tc.tile_pool(name="ep", bufs=2) as ep, \
             tc.tile_pool(name="otp", bufs=2) as otp:
            emap = {"A": nc.scalar, "D": nc.vector, "P": nc.gpsimd}
            sc_eng = mk_cycle(SC_PAT, emap)
            kv_eng = mk_cycle(KV_PAT, emap)
            vs_eng = mk_cycle(VS_PAT, emap)
            ksq_eng = mk_cycle(KSQ_PAT, emap)

            # ---- persistent SBUF state ----
            kTp = pers.tile([128, 2, n], F8, name="kTp", tag="kTp")
            qTp = pers.tile([128, 2, r], F8, name="qTp", tag="qTp")
            vp = [pers.tile([128, 2, D], F8, name=f"vp{jj}", tag=f"vp{jj}")
                  for jj in range(NJ // 2)]
            cs = pers.tile([128, NJ], F32, name="cs", tag="cs")
            cs8 = pers.tile([128, NJ, 1], F8, name="cs8", tag="cs8")
            wqt = pers.tile([128, 2, D], F8, name="wqt", tag="wqt")
            wkt = pers.tile([128, 2, D], F8, name="wkt", tag="wkt")
            wvt = pers.tile([128, 2, D], F8, name="wvt", tag="wvt")
            bq2 = pers.tile([128, 2], F32, name="bq2", tag="bq2")
            bk2 = pers.tile([128, 2], F32, name="bk2", tag="bk2")
            ones8 = pers.tile([128, 2, 1], F8, name="ones8", tag="ones8")
            ksf = pers.tile([128, 2, r], BF16, name="ksf", tag="ksf")
            qk8 = pers.tile([128, 2, r], F8, name="qk8", tag="qk8")
            qs8 = pers.tile([128, 2, r], F8, name="qs8", tag="qs8")
            vself = [pers.tile([128, 2, D], F32, name=f"vs{t}", tag=f"vs{t}")
                     for t in range(RT // 2)]
            xrt = [pers.tile([128, D], F32, name=f"xrs{t}", tag=f"xrs{t}")
                   for t in range(RT)]
            msb = pers.tile([128, RT], F32, name="msb", tag="msb")
            xrTp = pers.tile([128, 2, r], F8, name="xrTp", tag="xrTp")

            nc.gpsimd.dma_start(wkt[:], wk_d[:])
            nc.gpsimd.dma_start(wvt[:], wv_d[:])
            nc.gpsimd.dma_start(wqt[:], wq_d[:])
            nc.gpsimd.dma_start(xrTp[:], xrTp_d[:])
            nc.gpsimd.dma_start(bk2[:], bk_d[:])
            nc.gpsimd.dma_start(bq2[:], bq_d[:])
            nc.vector.memset(ones8[:], 1.0)
            epsb = pers.tile([128, 1], F32, name="epsb", tag="epsb")
            nc.vector.memset(epsb[:], 1e-24)

            def evac(eng, out_ap, in_ap, bias=None, scale=None, relu=False):
                """PSUM -> SBUF evacuation on ACT or DVE."""
                if eng is nc.scalar:
                    func = AF.Relu if relu else (AF.Identity if bias is not None
                                                 else AF.Copy)
                    kw = {}
                    if bias is not None:
                        kw["bias"] = bias
                    if scale is not None:
                        kw["scale"] = scale
                    nc.scalar.activation(out_ap, in_ap, func, **kw)
                else:
                    if relu:
                        if scale is not None:
                            eng.tensor_scalar(out=out_ap, in0=in_ap,
                                              scalar1=0.0, scalar2=scale,
                                              op0=ALU.max, op1=ALU.mult)
                        else:
                            eng.tensor_scalar(out=out_ap, in0=in_ap,
                                              scalar1=0.0, scalar2=None,
                                              op0=ALU.max)
                    elif bias is not None:
                        eng.tensor_scalar_add(out_ap, in_ap, bias)
                    elif scale is not None:
                        eng.tensor_scalar_mul(out_ap, in_ap, scale)
                    else:
                        eng.tensor_copy(out_ap, in_ap)

            xts = {}

            def fetch(ch, nsplit=1):
                if ch >= CH or ch in xts:
                    return
                xt = xtp.tile([128, 2, 1024], F8, name="xt", tag="xt")
                w = 1024 // nsplit
                for i in range(nsplit):
                    sl = slice(i * w, (i + 1) * w)
                    gl = slice(ch * 1024 + i * w, ch * 1024 + (i + 1) * w)
                    nc.sync.dma_start(xt[:, :, sl], xTp_d[:, :, gl])
                xts[ch] = xt

            def emit_chunk(ch, wp):
                """Stream one 1024-col slab of xTp; produce kTp, vp, cs."""
                xt = xts.pop(ch)
                fetch(ch + 1)
                crd = wp.tile([128, 8], F32, name="crd", tag="crd", bufs=2)
                for jh in range(2):
                    for db in range(2):
                        kps = wp.tile([128, 512], F32, name="kps", tag="w")
                        for m in range(2):
                            lsl = slice(jh * 512 + m * 256,
                                        jh * 512 + (m + 1) * 256)
                            nc.tensor.matmul(kps[:, m * 256:(m + 1) * 256],
                                             wkt[:, :, db * 128:(db + 1) * 128],
                                             xt[:, :, lsl],
                                             start=True, stop=True,
                                             perf_mode=PM.DoubleRow)
                        gsl = slice(ch * 1024 + jh * 512,
                                    ch * 1024 + (jh + 1) * 512)
                        evac(kv_eng(), kTp[:, db, gsl], kps[:],
                             bias=bk2[:, db:db + 1])
                    # ksq from the fp8 kT slab (SBUF), column norms via PE
                    gsl = slice(ch * 1024 + jh * 512, ch * 1024 + (jh + 1) * 512)
                    ksq = ksqp.tile([128, 2, 512], F8, name="ksq", tag="ksq")
                    ke = ksq_eng()
                    ke.tensor_tensor(out=ksq[:], in0=kTp[:, :, gsl],
                                     in1=kTp[:, :, gsl], op=ALU.mult)
                    for t in range(4):
                        col = jh * 4 + t
                        nc.tensor.matmul(crd[:, col:col + 1],
                                         ksq[:, :, t * 128:(t + 1) * 128],
                                         ones8[:],
                                         start=True, stop=True,
                                         perf_mode=PM.DoubleRow)
                    # cs for this half chunk (shortens the v-evac chain)
                    hs = slice(ch * 8 + jh * 4, ch * 8 + (jh + 1) * 4)
                    cl = slice(jh * 4, (jh + 1) * 4)
                    csn = ep.tile([128, 4], F32, name="csn", tag="csn")
                    nc.scalar.activation(csn[:], crd[:, cl], AF.Sqrt,
                                         bias=epsb[:])
                    nc.vector.reciprocal(cs[:, hs], csn[:])
                    nc.gpsimd.tensor_copy(cs8[:, hs, 0:1], cs[:, hs])
                # v projection: [j, d] psums, 2 j-blocks per bank;
                # pure evac, then per-jb cs scaling in SBUF (Pool-friendly)
                for pj in range(4):
                    jj = ch * 4 + pj
                    vps = wp.tile([128, 512], F32, name="vps", tag="w")
                    for i in range(2):
                        lsl = slice((pj * 2 + i) * 128, (pj * 2 + i + 1) * 128)
                        nc.tensor.matmul(vps[:, i * 256:(i + 1) * 256],
                                         xt[:, :, lsl], wvt[:],
                                         start=True, stop=True,
                                         perf_mode=PM.DoubleRow)
                    evac(kv_eng(), vp[jj][:], vps[:])
                    for i in range(2):
                        jb = 2 * jj + i
                        se = vs_eng()
                        if se is nc.scalar:
                            nc.scalar.activation(vp[jj][:, i, :],
                                                 vp[jj][:, i, :], AF.Copy,
                                                 scale=cs[:, jb:jb + 1])
                        else:
                            se.tensor_scalar(out=vp[jj][:, i, :],
                                             in0=vp[jj][:, i, :],
                                             scalar1=cs[:, jb:jb + 1],
                                             scalar2=None, op0=ALU.mult)

            def emit_rproj(wp):
                """q / k_self / v_self projections for this core's rows."""
                for db in range(2):
                    for rh in range(r // 512):
                        rsl = slice(rh * 512, (rh + 1) * 512)
                        qps = wp.tile([128, 512], F32, name="qps", tag="w")
                        for m in range(2):
                            msl = slice(rh * 512 + m * 256,
                                        rh * 512 + (m + 1) * 256)
                            nc.tensor.matmul(qps[:, m * 256:(m + 1) * 256],
                                             wqt[:, :, db * 128:(db + 1) * 128],
                                             xrTp[:, :, msl],
                                             start=True, stop=True,
                                             perf_mode=PM.DoubleRow)
                        evac(kv_eng(), qTp[:, db, rsl], qps[:],
                             bias=bq2[:, db:db + 1])
                        kps = wp.tile([128, 512], F32, name="ksps", tag="w")
                        for m in range(2):
                            msl = slice(rh * 512 + m * 256,
                                        rh * 512 + (m + 1) * 256)
                            nc.tensor.matmul(kps[:, m * 256:(m + 1) * 256],
                                             wkt[:, :, db * 128:(db + 1) * 128],
                                             xrTp[:, :, msl],
                                             start=True, stop=True,
                                             perf_mode=PM.DoubleRow)
                        evac(kv_eng(), ksf[:, db, rsl], kps[:],
                             bias=bk2[:, db:db + 1])
                for tt in range(RT // 2):
                    vsp = wp.tile([128, 512], F32, name="vsp", tag="w")
                    for i in range(2):
                        t = 2 * tt + i
                        nc.tensor.matmul(vsp[:, i * 256:(i + 1) * 256],
                                         xrTp[:, :, t * 128:(t + 1) * 128],
                                         wvt[:],
                                         start=True, stop=True,
                                         perf_mode=PM.DoubleRow)
                    evac(kv_eng(), vself[tt][:], vsp[:])

            def emit_selfterm(wp):
                """m = relu(diag(q.k_self)) / |k_self| for diagonal removal."""
                for h in range(2):
                    hsl = slice(h * 512, (h + 1) * 512)
                    nc.gpsimd.tensor_tensor(out=qk8[:, :, hsl],
                                            in0=qTp[:, :, hsl],
                                            in1=ksf[:, :, hsl], op=ALU.mult)
                    if QS_PAT[h % len(QS_PAT)] == "A":
                        nc.scalar.activation(qs8[:, :, hsl], ksf[:, :, hsl],
                                             AF.Square)
                    else:
                        nc.gpsimd.tensor_tensor(out=qs8[:, :, hsl],
                                                in0=ksf[:, :, hsl],
                                                in1=ksf[:, :, hsl],
                                                op=ALU.mult)
                sdkp = wp.tile([128, 2, RT], F32, name="sdkp", tag="crd",
                               bufs=2)
                for t in range(RT):
                    tsl = slice(t * 128, (t + 1) * 128)
                    nc.tensor.matmul(sdkp[:, 0, t:t + 1], qk8[:, :, tsl],
                                     ones8[:], start=True, stop=True,
                                     perf_mode=PM.DoubleRow)
                    nc.tensor.matmul(sdkp[:, 1, t:t + 1], qs8[:, :, tsl],
                                     ones8[:], start=True, stop=True,
                                     perf_mode=PM.DoubleRow)
                kst = ep.tile([128, RT], F32, name="kst", tag="kst")
                nc.scalar.activation(kst[:], sdkp[:, 1, :], AF.Sqrt,
                                     bias=epsb[:])
                inv = ep.tile([128, RT], F32, name="inv", tag="inv")
                nc.vector.reciprocal(inv[:], kst[:])
                nc.vector.tensor_scalar(out=msb[:], in0=sdkp[:, 0, :],
                                        scalar1=0.0, scalar2=None, op0=ALU.max)
                nc.gpsimd.tensor_tensor(out=msb[:], in0=msb[:], in1=inv[:],
                                        op=ALU.mult)

            def emit_scores(rb, g, sp, avm, avo):
                """4 j-blocks of scores -> relu -> fp8 wt4 -> AV accumulate."""
                rsl = slice(rb * RW, (rb + 1) * RW)
                sc = sp.tile([128, 1024], F32, name="sc", tag="sc")
                for i in range(GJB):
                    jb = g * GJB + i
                    nc.tensor.matmul(sc[:, i * 256:(i + 1) * 256],
                                     kTp[:, :, jb * 128:(jb + 1) * 128],
                                     qTp[:, :, rsl],
                                     start=True, stop=True,
                                     perf_mode=PM.DoubleRow)
                wt4 = wtp.tile([128, GJB, 256], F8, name="wt4", tag="wt4")
                evac(sc_eng(), wt4[:], sc[:], relu=True)
                for pj in range(GJB // 2):
                    jj = g * 2 + pj
                    for s in range(SS):
                        ssl = slice(s * 128, (s + 1) * 128)
                        nc.tensor.matmul(avm[:, s, :],
                                         wt4[:, 2 * pj:2 * pj + 2, ssl],
                                         vp[jj][:],
                                         start=(jj == 0),
                                         stop=(jj == NJ // 2 - 1),
                                         perf_mode=PM.DoubleRow)
                        nc.tensor.matmul(avo[:, s:s + 1],
                                         wt4[:, 2 * pj:2 * pj + 2, ssl],
                                         cs8[:, 2 * jj:2 * jj + 2, :],
                                         start=(jj == 0),
                                         stop=(jj == NJ // 2 - 1),
                                         perf_mode=PM.DoubleRow)

            def emit_epilogue(rb, avm, avo):
                avv = ep.tile([128, SS, 256], F32, name="avv", tag="avv")
                nc.scalar.activation(avv[:], avm[:], AF.Copy)
                dens = ep.tile([128, SS], F32, name="dens", tag="dens")
                nc.vector.tensor_scalar_add(dens[:], avo[:], 1e-9)
                for s in range(SS):
                    t = rb * SS + s
                    tmp = ep.tile([128, D], F32, name="tmp", tag="tmp")
                    nc.gpsimd.tensor_scalar(out=tmp[:],
                                            in0=vself[t // 2][:, t % 2, :],
                                            scalar1=msb[:, t:t + 1],
                                            scalar2=None, op0=ALU.mult)
                    num2 = ep.tile([128, D], F32, name="num2", tag="num2")
                    nc.gpsimd.tensor_tensor(out=num2[:], in0=avv[:, s, :],
                                            in1=tmp[:], op=ALU.subtract)
                    den = ep.tile([128, 1], F32, name="den", tag="den")
                    nc.vector.tensor_scalar(out=den[:], in0=dens[:, s:s + 1],
                                            scalar1=msb[:, t:t + 1],
                                            scalar2=None, op0=ALU.subtract)
                    rec = ep.tile([128, 1], F32, name="rec", tag="rec")
                    nc.vector.reciprocal(rec[:], den[:])
                    ot1 = ep.tile([128, D], F32, name="ot1", tag="ot1")
                    nc.gpsimd.tensor_scalar(out=ot1[:], in0=num2[:],
                                            scalar1=rec[:], scalar2=None,
                                            op0=ALU.mult)
                    ot = otp.tile([128, D], F32, name="ot", tag="ot")
                    nc.gpsimd.tensor_tensor(out=ot[:], in0=ot1[:],
                                            in1=xrt[t][:], op=ALU.add)
                    nc.sync.dma_start(out_d[t * 128:(t + 1) * 128, :], ot[:])

            # ---- phase A: all projections (PSUM: 2 k/q banks + 2x2 v
            # banks + 2 colsum banks) ----
            with tc.tile_pool(name="wp", bufs=6, space="PSUM") as wp:
                fetch(0, nsplit=4)
                emit_chunk(0, wp)
                emit_rproj(wp)
                emit_selfterm(wp)
                for t in range(RT):
                    nc.gpsimd.dma_start(xrt[t][:],
                                        xr_d[t * 128:(t + 1) * 128, :])
                for ch in range(1, CH):
                    emit_chunk(ch, wp)

            # ---- phase B: scores + w@v (3 x 2-bank scores + 2 accum) ----
            with tc.tile_pool(name="sp", bufs=3, space="PSUM") as sp, \
                 tc.tile_pool(name="avp", bufs=1, space="PSUM") as avp:
                for rb in range(NRB):
                    avm = avp.tile([128, SS, 256], F32, name="avm", tag="avm")
                    avo = avp.tile([128, SS], F32, name="avo", tag="avo")
                    for g in range(NG):
                        emit_scores(rb, g, sp, avm, avo)
                    emit_epilogue(rb, avm, avo)
    nc.compile()
    return nc


def _get_nc(n=N, r=N // M):
    key = (n, r)
    if key not in _CACHE:
        _CACHE[key] = build(n, r)
    return _CACHE[key]


def _pairT(a2d):
    """[n, 256] -> fp8 pair layout [128, 2, n] (transposed)."""
    f8 = mybir.dt.np(F8)
    a = np.asarray(a2d, np.float32).astype(f8)
    n = a.shape[0]
    return np.ascontiguousarray(a.T.reshape(2, 128, n).transpose(1, 0, 2))


def kernel(x, Wq, bq, Wk, bk, Wv, bv):
    global LAST
    x = np.ascontiguousarray(np.asarray(x, np.float32))
    n = x.shape[0]
    r = n // M
    xTp = _pairT(x)
    wqTp = _pairT(np.asarray(Wq, np.float32))   # == Wq.T in pair layout
    wkTp = _pairT(np.asarray(Wk, np.float32))
    wvTp = _pairT(np.asarray(Wv, np.float32))
    xplus = x + np.asarray(bv, np.float32)[None, :]
    bq2 = np.ascontiguousarray(np.asarray(bq, np.float32).reshape(2, 128).T)
    bk2 = np.ascontiguousarray(np.asarray(bk, np.float32).reshape(2, 128).T)
    in_maps = []
    for c in range(M):
        rows = slice(c * r, (c + 1) * r)
        in_maps.append({
            "xTp": xTp,
            "xrTp": _pairT(x[rows]),
            "xr": np.ascontiguousarray(xplus[rows]),
            "wqTp": wqTp, "wkTp": wkTp, "wvTp": wvTp,
            "bq2": bq2, "bk2": bk2,
        })
    res = run_bass_kernel_spmd(_get_nc(n, r), in_maps, core_ids=list(range(M)),
                               trace=TRACE)
    LAST = res
    return np.concatenate([res.results[c]["out"] for c in range(M)], axis=0)


# revision 39
# speedup vs baseline: 2.0367x; 1.0123x over previous
"""Trainium2 Bass kernel for nn_AttentionBlock (sparse_attention).

Reference computation (N=8192, D=256):
    q = l2norm(x @ Wq.T + bq); k = l2norm(x @ Wk.T + bk); v = x @ Wv.T + bv
    w = relu(q @ k.T); w[diag] = 0; w /= max(rowsum(w), eps)
    out = w @ v + x

Algebraic restructuring (same as the bf16 baseline):
  * relu is positively homogeneous and rows are renormalized by their sum,
    so the q-normalization scale cancels: skip it entirely.
  * The k-normalization column scale cs_j = 1/|k_j| commutes through relu.
    It is folded into v (v rows scaled by cs_j at the v evacuation) and
    carried as an fp8 copy of cs for the denominator row sums
    (flash-attention ones-trick, with cs8 as the moving operand).
  * The zeroed diagonal is handled by subtracting a separately computed self
    term m_r = relu(q_r . k_r)/|k_r| from numerator (m_r * v_r) and row sum.
  * v bias folded into the host-side residual (xr + bv).

Speed: all big matmuls run in fp8 (e4m3) with MatmulPerfMode.DoubleRow:
one matmul contracts 2x128 partitions at 0.5 cycles/row, 4x fewer PE
cycles than bf16 pairing for the same D=256 contraction.  fp8 operands
live in "pair layout" [128, 2, n]: partition p, pair i holds contraction
element i*128+p.  Scores for 4 j-blocks accumulate in one 2-bank PSUM
tile [128, 1024] and leave through a single pure-relu evacuation into
wt4 [128, 4, 256] fp8, which directly exposes the DoubleRow stationary
pairs for the w @ v matmul.

The PSUM evacuations (relu on scores, bias-adds on k/q, cs-scaled copies
of v) are the bottleneck; only DVE and ACT can read PSUM, so they split
that work while GPSIMD (Pool) takes SBUF-side work (k^2 for the column
norms, q*k self-term products, epilogue arithmetic).  The kernel runs in
two sequential phases so each gets the PSUM banks it needs: phase A
(projections; 6 rotating work banks) then phase B (scores + w@v; 3
double-bank score tiles + 2 accumulator banks).

Numerics (numpy emulation of the full fp8 pipeline): rel err ~2.7e-3
vs the fp32 reference, comfortably under the 2e-2 gate.
"""

import numpy as np

import concourse.bass as bass
import concourse.bacc as bacc
import concourse.mybir as mybir
from concourse import tile
from concourse.bass_utils import run_bass_kernel_spmd

F32 = mybir.dt.float32
BF16 = mybir.dt.bfloat16
F8 = mybir.dt.float8e4
AF = mybir.ActivationFunctionType
PM = mybir.MatmulPerfMode
ALU = mybir.AluOpType

M = 8       # cores
N = 8192    # tokens
D = 256     # feature dim

TRACE = False
LAST = None
_CACHE = {}

# engine split knobs (tuned against the scheduler makespan)
import os
SC_PAT = os.environ.get("K_SC", "DA")
KV_PAT = os.environ.get("K_KV", "AD")    # k/q/ksf/vself/v psum evacs
VS_PAT = os.environ.get("K_VS", "P")     # in-place cs scaling of v (SBUF)
KSQ_PAT = os.environ.get("K_KSQ", "PDPP")  # ksq: P=Pool, D=DVE
QS_PAT = os.environ.get("K_QS", "A")     # qs squares: A=ACT, P=Pool


def build(n=N, r=N // M):
    NJ = n // 128            # 64  j blocks
    CH = n // 1024           # 8   xT streaming chunks (1024 j each)
    RT = r // 128            # 8   128-row tiles of this core's rows
    RW = 256                 # row-block width (r cols per score group)
    NRB = r // RW            # 4   row blocks
    SS = RW // 128           # 2   128-row subtiles per row block
    GJB = 4                  # j blocks per score psum group
    NG = NJ // GJB           # 16  score groups per row block

    nc = bacc.Bacc(None)
    xTp_d = nc.declare_dram_parameter("xTp", [128, 2, n], F8, isOutput=False)
    xrTp_d = nc.declare_dram_parameter("xrTp", [128, 2, r], F8, isOutput=False)
    xr_d = nc.declare_dram_parameter("xr", [r, D], F32, isOutput=False)
    wq_d = nc.declare_dram_parameter("wqTp", [128, 2, D], F8, isOutput=False)
    wk_d = nc.declare_dram_parameter("wkTp", [128, 2, D], F8, isOutput=False)
    wv_d = nc.declare_dram_parameter("wvTp", [128, 2, D], F8, isOutput=False)
    bq_d = nc.declare_dram_parameter("bq2", [128, 2], F32, isOutput=False)
    bk_d = nc.declare_dram_parameter("bk2", [128, 2], F32, isOutput=False)
    out_d = nc.declare_dram_parameter("out", [r, D], F32, isOutput=True)

    def mk_cycle(pat, m):
        state = {"i": 0}
        def nxt():
            e = m[pat[state["i"] % len(pat)]]
            state["i"] += 1
            return e
        return nxt

    with tile.TileContext(nc, pool_alloc_mode="queue") as tc:
        B = lambda k, d: int(os.environ.get(k, d))
        with tc.tile_pool(name="pers", bufs=1) as pers, \
             tc.tile_pool(name="xtp", bufs=B("K_XT", 2)) as xtp, \
             tc.tile_pool(name="ksqp", bufs=B("K_KSQB", 2)) as ksqp, \
             tc.tile_pool(name="wtp", bufs=B("K_WT", 4)) as wtp, \
             tc.tile_pool(name="ep", bufs=B("K_EP", 2)) as ep, \
             tc.tile_pool(name="otp", bufs=2) as otp:
            emap = {"A": nc.scalar, "D": nc.vector, "P": nc.gpsimd}
            sc_eng = mk_cycle(SC_PAT, emap)
            kv_eng = mk_cycle(KV_PAT, emap)
            vs_eng = mk_cycle(VS_PAT, emap)
            ksq_eng = mk_cycle(KSQ_PAT, emap)

            # ---- persistent SBUF state ----
            kTp = pers.tile([128, 2, n], F8, name="kTp", tag="kTp")
            qTp = pers.tile([128, 2, r], F8, name="qTp", tag="qTp")
            vp = [pers.tile([128, 2, D], F8, name=f"vp{jj}", tag=f"vp{jj}")
                  for jj in range(NJ // 2)]
            cs = pers.tile([128, NJ], F32, name="cs", tag="cs")
            cs8 = pers.tile([128, NJ, 1], F8, name="cs8", tag="cs8")
            wqt = pers.tile([128, 2, D], F8, name="wqt", tag="wqt")
            wkt = pers.tile([128, 2, D], F8, name="wkt", tag="wkt")
            wvt = pers.tile([128, 2, D], F8, name="wvt", tag="wvt")
            bq2 = pers.tile([128, 2], F32, name="bq2", tag="bq2")
            bk2 = pers.tile([128, 2], F32, name="bk2", tag="bk2")
            ones8 = pers.tile([128, 2, 1], F8, name="ones8", tag="ones8")
            ksf = pers.tile([128, 2, r], BF16, name="ksf", tag="ksf")
            qk8 = pers.tile([128, 2, r], F8, name="qk8", tag="qk8")
            qs8 = pers.tile([128, 2, r], F8, name="qs8", tag="qs8")
            vself = [pers.tile([128, 2, D], F32, name=f"vs{t}", tag=f"vs{t}")
                     for t in range(RT // 2)]
            xrt = [pers.tile([128, D], F32, name=f"xrs{t}", tag=f"xrs{t}")
                   for t in range(RT)]
            msb = pers.tile([128, RT], F32, name="msb", tag="msb")
            xrTp = pers.tile([128, 2, r], F8, name="xrTp", tag="xrTp")

            nc.gpsimd.dma_start(wkt[:], wk_d[:])
            nc.gpsimd.dma_start(wvt[:], wv_d[:])
            nc.gpsimd.dma_start(wqt[:], wq_d[:])
            nc.gpsimd.dma_start(xrTp[:], xrTp_d[:])
            nc.gpsimd.dma_start(bk2[:], bk_d[:])
            nc.gpsimd.dma_start(bq2[:], bq_d[:])
            nc.vector.memset(ones8[:], 1.0)
            epsb = pers.tile([128, 1], F32, name="epsb", tag="epsb")
            nc.vector.memset(epsb[:], 1e-24)

            def evac(eng, out_ap, in_ap, bias=None, scale=None, relu=False):
                """PSUM -> SBUF evacuation on ACT or DVE."""
                if eng is nc.scalar:
                    func = AF.Relu if relu else (AF.Identity if bias is not None
                                                 else AF.Copy)
                    kw = {}
                    if bias is not None:
                        kw["bias"] = bias
                    if scale is not None:
                        kw["scale"] = scale
                    nc.scalar.activation(out_ap, in_ap, func, **kw)
                else:
                    if relu:
                        if scale is not None:
                            eng.tensor_scalar(out=out_ap, in0=in_ap,
                                              scalar1=0.0, scalar2=scale,
                                              op0=ALU.max, op1=ALU.mult)
                        else:
                            eng.tensor_scalar(out=out_ap, in0=in_ap,
                                              scalar1=0.0, scalar2=None,
                                              op0=ALU.max)
                    elif bias is not None:
                        eng.tensor_scalar_add(out_ap, in_ap, bias)
                    elif scale is not None:
                        eng.tensor_scalar_mul(out_ap, in_ap, scale)
                    else:
                        eng.tensor_copy(out_ap, in_ap)

            xts = {}

            def fetch(ch, nsplit=1):
                if ch >= CH or ch in xts:
                    return
                xt = xtp.tile([128, 2, 1024], F8, name="xt", tag="xt")
                w = 1024 // nsplit
                for i in range(nsplit):
                    sl = slice(i * w, (i + 1) * w)
                    gl = slice(ch * 1024 + i * w, ch * 1024 + (i + 1) * w)
                    nc.sync.dma_start(xt[:, :, sl], xTp_d[:, :, gl])
                xts[ch] = xt

            def emit_chunk(ch, wp):
                """Stream one 1024-col slab of xTp; produce kTp, vp, cs."""
                xt = xts.pop(ch)
                fetch(ch + 1)
                crd = wp.tile([128, 8], F32, name="crd", tag="crd", bufs=2)
                for jh in range(2):
                    for db in range(2):
                        kps = wp.tile([128, 512], F32, name="kps", tag="w")
                        for m in range(2):
                            lsl = slice(jh * 512 + m * 256,
                                        jh * 512 + (m + 1) * 256)
                            nc.tensor.matmul(kps[:, m * 256:(m + 1) * 256],
                                             wkt[:, :, db * 128:(db + 1) * 128],
                                             xt[:, :, lsl],
                                             start=True, stop=True,
                                             perf_mode=PM.DoubleRow)
                        gsl = slice(ch * 1024 + jh * 512,
                                    ch * 1024 + (jh + 1) * 512)
                        evac(kv_eng(), kTp[:, db, gsl], kps[:],
                             bias=bk2[:, db:db + 1])
                    # ksq from the fp8 kT slab (SBUF), column norms via PE
                    gsl = slice(ch * 1024 + jh * 512, ch * 1024 + (jh + 1) * 512)
                    ksq = ksqp.tile([128, 2, 512], F8, name="ksq", tag="ksq")
                    ke = ksq_eng()
                    ke.tensor_tensor(out=ksq[:], in0=kTp[:, :, gsl],
                                     in1=kTp[:, :, gsl], op=ALU.mult)
                    for t in range(4):
                        col = jh * 4 + t
                        nc.tensor.matmul(crd[:, col:col + 1],
                                         ksq[:, :, t * 128:(t + 1) * 128],
                                         ones8[:],
                                         start=True, stop=True,
                                         perf_mode=PM.DoubleRow)
                    # cs for this half chunk (shortens the v-evac chain)
                    hs = slice(ch * 8 + jh * 4, ch * 8 + (jh + 1) * 4)
                    cl = slice(jh * 4, (jh + 1) * 4)
                    csn = ep.tile([128, 4], F32, name="csn", tag="csn")
                    nc.scalar.activation(csn[:], crd[:, cl], AF.Sqrt,
                                         bias=epsb[:])
                    nc.vector.reciprocal(cs[:, hs], csn[:])
                    nc.gpsimd.tensor_copy(cs8[:, hs, 0:1], cs[:, hs])
                # v projection: [j, d] psums, 2 j-blocks per bank;
                # pure evac, then per-jb cs scaling in SBUF (Pool-friendly)
                for pj in range(4):
                    jj = ch * 4 + pj
                    vps = wp.tile([128, 512], F32, name="vps", tag="w")
                    for i in range(2):
                        lsl = slice((pj * 2 + i) * 128, (pj * 2 + i + 1) * 128)
                        nc.tensor.matmul(vps[:, i * 256:(i + 1) * 256],
                                         xt[:, :, lsl], wvt[:],
                                         start=True, stop=True,
                                         perf_mode=PM.DoubleRow)
                    evac(kv_eng(), vp[jj][:], vps[:])
                    for i in range(2):
                        jb = 2 * jj + i
                        se = vs_eng()
                        if se is nc.scalar:
                            nc.scalar.activation(vp[jj][:, i, :],
                                                 vp[jj][:, i, :], AF.Copy,
                                                 scale=cs[:, jb:jb + 1])
                        else:
                            se.tensor_scalar(out=vp[jj][:, i, :],
                                             in0=vp[jj][:, i, :],
                                             scalar1=cs[:, jb:jb + 1],
                                             scalar2=None, op0=ALU.mult)

            def emit_rproj(wp):
                """q / k_self / v_self projections for this core's rows."""
                for db in range(2):
                    for rh in range(r // 512):
                        rsl = slice(rh * 512, (rh + 1) * 512)
                        qps = wp.tile([128, 512], F32, name="qps", tag="w")
                        for m in range(2):
                            msl = slice(rh * 512 + m * 256,
                                        rh * 512 + (m + 1) * 256)
                            nc.tensor.matmul(qps[:, m * 256:(m + 1) * 256],
                                             wqt[:, :, db * 128:(db + 1) * 128],
                                             xrTp[:, :, msl],
                                             start=True, stop=True,
                                             perf_mode=PM.DoubleRow)
                        evac(kv_eng(), qTp[:, db, rsl], qps[:],
                             bias=bq2[:, db:db + 1])
                        kps = wp.tile([128, 512], F32, name="ksps", tag="w")
                        for m in range(2):
                            msl = slice(rh * 512 + m * 256,
                                        rh * 512 + (m + 1) * 256)
                            nc.tensor.matmul(kps[:, m * 256:(m + 1) * 256],
                                             wkt[:, :, db * 128:(db + 1) * 128],
                                             xrTp[:, :, msl],
                                             start=True, stop=True,
                                             perf_mode=PM.DoubleRow)
                        evac(kv_eng(), ksf[:, db, rsl], kps[:],
                             bias=bk2[:, db:db + 1])
                for tt in range(RT // 2):
                    vsp = wp.tile([128, 512], F32, name="vsp", tag="w")
                    for i in range(2):
                        t = 2 * tt + i
                        nc.tensor.matmul(vsp[:, i * 256:(i + 1) * 256],
                                         xrTp[:, :, t * 128:(t + 1) * 128],
                                         wvt[:],
                                         start=True, stop=True,
                                         perf_mode=PM.DoubleRow)
                    evac(kv_eng(), vself[tt][:], vsp[:])

            def emit_selfterm(wp):
                """m = relu(diag(q.k_self)) / |k_self| for diagonal removal."""
                for h in range(2):
                    hsl = slice(h * 512, (h + 1) * 512)
                    nc.gpsimd.tensor_tensor(out=qk8[:, :, hsl],
                                            in0=qTp[:, :, hsl],
                                            in1=ksf[:, :, hsl], op=ALU.mult)
                    if QS_PAT[h % len(QS_PAT)] == "A":
                        nc.scalar.activation(qs8[:, :, hsl], ksf[:, :, hsl],
                                             AF.Square)
                    else:
                        nc.gpsimd.tensor_tensor(out=qs8[:, :, hsl],
                                                in0=ksf[:, :, hsl],
                                                in1=ksf[:, :, hsl],
                                                op=ALU.mult)
                sdkp = wp.tile([128, 2, RT], F32, name="sdkp", tag="crd",
                               bufs=2)
                for t in range(RT):
                    tsl = slice(t * 128, (t + 1) * 128)
                    nc.tensor.matmul(sdkp[:, 0, t:t + 1], qk8[:, :, tsl],
                                     ones8[:], start=True, stop=True,
                                     perf_mode=PM.DoubleRow)
                    nc.tensor.matmul(sdkp[:, 1, t:t + 1], qs8[:, :, tsl],
                                     ones8[:], start=True, stop=True,
                                     perf_mode=PM.DoubleRow)
                kst = ep.tile([128, RT], F32, name="kst", tag="kst")
                nc.scalar.activation(kst[:], sdkp[:, 1, :], AF.Sqrt,
                                     bias=epsb[:])
                inv = ep.tile([128, RT], F32, name="inv", tag="inv")
                nc.vector.reciprocal(inv[:], kst[:])
                nc.vector.tensor_scalar(out=msb[:], in0=sdkp[:, 0, :],
                                        scalar1=0.0, scalar2=None, op0=ALU.max)
                nc.gpsimd.tensor_tensor(out=msb[:], in0=msb[:], in1=inv[:],
                                        op=ALU.mult)

            def emit_scores(rb, g, sp, avm, avo):
                """4 j-blocks of scores -> relu -> fp8 wt4 -> AV accumulate."""
                rsl = slice(rb * RW, (rb + 1) * RW)
                sc = sp.tile([128, 1024], F32, name="sc", tag="sc")
                for i in range(GJB):
                    jb = g * GJB + i
                    nc.tensor.matmul(sc[:, i * 256:(i + 1) * 256],
                                     kTp[:, :, jb * 128:(jb + 1) * 128],
                                     qTp[:, :, rsl],
                                     start=True, stop=True,
                                     perf_mode=PM.DoubleRow)
                wt4 = wtp.tile([128, GJB, 256], F8, name="wt4", tag="wt4")
                evac(sc_eng(), wt4[:], sc[:], relu=True)
                for pj in range(GJB // 2):
                    jj = g * 2 + pj
                    for s in range(SS):
                        ssl = slice(s * 128, (s + 1) * 128)
                        nc.tensor.matmul(avm[:, s, :],
                                         wt4[:, 2 * pj:2 * pj + 2, ssl],
                                         vp[jj][:],
                                         start=(jj == 0),
                                         stop=(jj == NJ // 2 - 1),
                                         perf_mode=PM.DoubleRow)
                        nc.tensor.matmul(avo[:, s:s + 1],
                                         wt4[:, 2 * pj:2 * pj + 2, ssl],
                                         cs8[:, 2 * jj:2 * jj + 2, :],
                                         start=(jj == 0),
                                         stop=(jj == NJ // 2 - 1),
                                         perf_mode=PM.DoubleRow)

            def emit_epilogue(rb, avm, avo):
                avv = ep.tile([128, SS, 256], F32, name="avv", tag="avv")
                nc.scalar.activation(avv[:], avm[:], AF.Copy)
                dens = ep.tile([128, SS], F32, name="dens", tag="dens")
                nc.vector.tensor_scalar_add(dens[:], avo[:], 1e-9)
                for s in range(SS):
                    t = rb * SS + s
                    tmp = ep.tile([128, D], F32, name="tmp", tag="tmp")
                    nc.gpsimd.tensor_scalar(out=tmp[:],
                                            in0=vself[t // 2][:, t % 2, :],
                                            scalar1=msb[:, t:t + 1],
                                            scalar2=None, op0=ALU.mult)
                    num2 = ep.tile([128, D], F32, name="num2", tag="num2")
                    nc.gpsimd.tensor_tensor(out=num2[:], in0=avv[:, s, :],
                                            in1=tmp[:], op=ALU.subtract)
                    den = ep.tile([128, 1], F32, name="den", tag="den")
                    nc.vector.tensor_scalar(out=den[:], in0=dens[:, s:s + 1],
                                            scalar1=msb[:, t:t + 1],
                                            scalar2=None, op0=ALU.subtract)
                    rec = ep.tile([128, 1], F32, name="rec", tag="rec")
                    nc.vector.reciprocal(rec[:], den[:])
                    ot1 = ep.tile([128, D], F32, name="ot1", tag="ot1")
                    nc.gpsimd.tensor_scalar(out=ot1[:], in0=num2[:],
                                            scalar1=rec[:], scalar2=None,
                                            op0=ALU.mult)
                    ot = otp.tile([128, D], F32, name="ot", tag="ot")
                    nc.gpsimd.tensor_tensor(out=ot[:], in0=ot1[:],
                                            in1=xrt[t][:], op=ALU.add)
                    nc.sync.dma_start(out_d[t * 128:(t + 1) * 128, :], ot[:])

            # ---- phase A: all projections (PSUM: 2 k/q banks + 2x2 v
            # banks + 2 colsum banks) ----
            with tc.tile_pool(name="wp", bufs=6, space="PSUM") as wp:
                fetch(0, nsplit=4)
                emit_chunk(0, wp)
                emit_rproj(wp)
                emit_selfterm(wp)
                for t in range(RT):
                    nc.gpsimd.dma_start(xrt[t][:],
                                        xr_d[t * 128:(t + 1) * 128, :])
                for ch in range(1, CH):
                    emit_chunk(ch, wp)

            # ---- phase B: scores + w@v (3 x 2-bank scores + 2 accum) ----
            with tc.tile_pool(name="sp", bufs=3, space="PSUM") as sp, \
                 tc.tile_pool(name="avp", bufs=1, space="PSUM") as avp:
                for rb in range(NRB):
                    avm = avp.tile([128, SS, 256], F32, name="avm", tag="avm")
                    avo = avp.tile([128, SS], F32, name="avo", tag="avo")
                    for g in range(NG):
                        emit_scores(rb, g, sp, avm, avo)
                    emit_epilogue(rb, avm, avo)
    nc.compile()
    return nc


def _get_nc(n=N, r=N // M):
    key = (n, r)
    if key not in _CACHE:
        _CACHE[key] = build(n, r)
    return _CACHE[key]


def _pairT(a2d):
    """[n, 256] -> fp8 pair layout [128, 2, n] (transposed)."""
    f8 = mybir.dt.np(F8)
    a = np.asarray(a2d, np.float32).astype(f8)
    n = a.shape[0]
    return np.ascontiguousarray(a.T.reshape(2, 128, n).transpose(1, 0, 2))


def kernel(x, Wq, bq, Wk, bk, Wv, bv):
    global LAST
    x = np.ascontiguousarray(np.asarray(x, np.float32))
    n = x.shape[0]
    r = n // M
    xTp = _pairT(x)
    wqTp = _pairT(np.asarray(Wq, np.float32))   # == Wq.T in pair layout
    wkTp = _pairT(np.asarray(Wk, np.float32))
    wvTp = _pairT(np.asarray(Wv, np.float32))
    xplus = x + np.asarray(bv, np.float32)[None, :]
    bq2 = np.ascontiguousarray(np.asarray(bq, np.float32).reshape(2, 128).T)
    bk2 = np.ascontiguousarray(np.asarray(bk, np.float32).reshape(2, 128).T)
    in_maps = []
    for c in range(M):
        rows = slice(c * r, (c + 1) * r)
        in_maps.append({
            "xTp": xTp,
            "xrTp": _pairT(x[rows]),
            "xr": np.ascontiguousarray(xplus[rows]),
            "wqTp": wqTp, "wkTp": wkTp, "wvTp": wvTp,
            "bq2": bq2, "bk2": bk2,
        })
    res = run_bass_kernel_spmd(_get_nc(n, r), in_maps, core_ids=list(range(M)),
                               trace=TRACE)
    LAST = res
    return np.concatenate([res.results[c]["out"] for c in range(M)], axis=0)


# revision 40
# speedup vs baseline: 2.0414x; 1.0023x over previous
"""Trainium2 Bass kernel for nn_AttentionBlock (sparse_attention).

Reference computation (N=8192, D=256):
    q = l2norm(x @ Wq.T + bq); k = l2norm(x @ Wk.T + bk); v = x @ Wv.T + bv
    w = relu(q @ k.T); w[diag] = 0; w /= max(rowsum(w), eps)
    out = w @ v + x

Algebraic restructuring (same as the bf16 baseline):
  * relu is positively homogeneous and rows are renormalized by their sum,
    so the q-normalization scale cancels: skip it entirely.
  * The k-normalization column scale cs_j = 1/|k_j| commutes through relu.
    It is folded into v (v rows scaled by cs_j at the v evacuation) and
    carried as an fp8 copy of cs for the denominator row sums
    (flash-attention ones-trick, with cs8 as the moving operand).
  * The zeroed diagonal is handled by subtracting a separately computed self
    term m_r = relu(q_r . k_r)/|k_r| from numerator (m_r * v_r) and row sum.
  * v bias folded into the host-side residual (xr + bv).

Speed: all big matmuls run in fp8 (e4m3) with MatmulPerfMode.DoubleRow:
one matmul contracts 2x128 partitions at 0.5 cycles/row, 4x fewer PE
cycles than bf16 pairing for the same D=256 contraction.  fp8 operands
live in "pair layout" [128, 2, n]: partition p, pair i holds contraction
element i*128+p.  Scores for 4 j-blocks accumulate in one 2-bank PSUM
tile [128, 1024] and leave through a single pure-relu evacuation into
wt4 [128, 4, 256] fp8, which directly exposes the DoubleRow stationary
pairs for the w @ v matmul.

The PSUM evacuations (relu on scores, bias-adds on k/q, cs-scaled copies
of v) are the bottleneck; only DVE and ACT can read PSUM, so they split
that work while GPSIMD (Pool) takes SBUF-side work (k^2 for the column
norms, q*k self-term products, epilogue arithmetic).  The kernel runs in
two sequential phases so each gets the PSUM banks it needs: phase A
(projections; 6 rotating work banks) then phase B (scores + w@v; 3
double-bank score tiles + 2 accumulator banks).

Numerics (numpy emulation of the full fp8 pipeline): rel err ~2.7e-3
vs the fp32 reference, comfortably under the 2e-2 gate.
"""

import numpy as np

import concourse.bass as bass
import concourse.bacc as bacc
import concourse.mybir as mybir
from concourse import tile
from concourse.bass_utils import run_bass_kernel_spmd

F32 = mybir.dt.float32
BF16 = mybir.dt.bfloat16
F8 = mybir.dt.float8e4
AF = mybir.ActivationFunctionType
PM = mybir.MatmulPerfMode
ALU = mybir.AluOpType

M = 8       # cores
N = 8192    # tokens
D = 256     # feature dim

TRACE = False
LAST = None
_CACHE = {}

# engine split knobs (tuned against the scheduler makespan)
import os
SC_PAT = os.environ.get("K_SC", "DA")
KV_PAT = os.environ.get("K_KV", "DA")    # k/q/ksf/vself/v psum evacs
VS_PAT = os.environ.get("K_VS", "P")     # in-place cs scaling of v (SBUF)
KSQ_PAT = os.environ.get("K_KSQ", "PDPP")  # ksq: P=Pool, D=DVE
QS_PAT = os.environ.get("K_QS", "A")     # qs squares: A=ACT, P=Pool


def build(n=N, r=N // M):
    NJ = n // 128            # 64  j blocks
    CH = n // 1024           # 8   xT streaming chunks (1024 j each)
    RT = r // 128            # 8   128-row tiles of this core's rows
    RW = 256                 # row-block width (r cols per score group)
    NRB = r // RW            # 4   row blocks
    SS = RW // 128           # 2   128-row subtiles per row block
    GJB = 4                  # j blocks per score psum group
    NG = NJ // GJB           # 16  score groups per row block

    nc = bacc.Bacc(None)
    xTp_d = nc.declare_dram_parameter("xTp", [128, 2, n], F8, isOutput=False)
    xrTp_d = nc.declare_dram_parameter("xrTp", [128, 2, r], F8, isOutput=False)
    xr_d = nc.declare_dram_parameter("xr", [r, D], F32, isOutput=False)
    wq_d = nc.declare_dram_parameter("wqTp", [128, 2, D], F8, isOutput=False)
    wk_d = nc.declare_dram_parameter("wkTp", [128, 2, D], F8, isOutput=False)
    wv_d = nc.declare_dram_parameter("wvTp", [128, 2, D], F8, isOutput=False)
    bq_d = nc.declare_dram_parameter("bq2", [128, 2], F32, isOutput=False)
    bk_d = nc.declare_dram_parameter("bk2", [128, 2], F32, isOutput=False)
    out_d = nc.declare_dram_parameter("out", [r, D], F32, isOutput=True)

    def mk_cycle(pat, m):
        state = {"i": 0}
        def nxt():
            e = m[pat[state["i"] % len(pat)]]
            state["i"] += 1
            return e
        return nxt

    with tile.TileContext(nc, pool_alloc_mode="queue") as tc:
        B = lambda k, d: int(os.environ.get(k, d))
        with tc.tile_pool(name="pers", bufs=1) as pers, \
             tc.tile_pool(name="xtp", bufs=B("K_XT", 2)) as xtp, \
             tc.tile_pool(name="ksqp", bufs=B("K_KSQB", 2)) as ksqp, \
             tc.tile_pool(name="wtp", bufs=B("K_WT", 4)) as wtp, \
             tc.tile_pool(name="ep", bufs=B("K_EP", 2)) as ep, \
             tc.tile_pool(name="otp", bufs=2) as otp:
            emap = {"A": nc.scalar, "D": nc.vector, "P": nc.gpsimd}
            sc_eng = mk_cycle(SC_PAT, emap)
            kv_eng = mk_cycle(KV_PAT, emap)
            vs_eng = mk_cycle(VS_PAT, emap)
            ksq_eng = mk_cycle(KSQ_PAT, emap)

            # ---- persistent SBUF state ----
            kTp = pers.tile([128, 2, n], F8, name="kTp", tag="kTp")
            qTp = pers.tile([128, 2, r], F8, name="qTp", tag="qTp")
            vp = [pers.tile([128, 2, D], F8, name=f"vp{jj}", tag=f"vp{jj}")
                  for jj in range(NJ // 2)]
            cs = pers.tile([128, NJ], F32, name="cs", tag="cs")
            cs8 = pers.tile([128, NJ, 1], F8, name="cs8", tag="cs8")
            wqt = pers.tile([128, 2, D], F8, name="wqt", tag="wqt")
            wkt = pers.tile([128, 2, D], F8, name="wkt", tag="wkt")
            wvt = pers.tile([128, 2, D], F8, name="wvt", tag="wvt")
            bq2 = pers.tile([128, 2], F32, name="bq2", tag="bq2")
            bk2 = pers.tile([128, 2], F32, name="bk2", tag="bk2")
            ones8 = pers.tile([128, 2, 1], F8, name="ones8", tag="ones8")
            ksf = pers.tile([128, 2, r], BF16, name="ksf", tag="ksf")
            qk8 = pers.tile([128, 2, r], F8, name="qk8", tag="qk8")
            qs8 = pers.tile([128, 2, r], F8, name="qs8", tag="qs8")
            vself = [pers.tile([128, 2, D], F32, name=f"vs{t}", tag=f"vs{t}")
                     for t in range(RT // 2)]
            xrt = [pers.tile([128, D], F32, name=f"xrs{t}", tag=f"xrs{t}")
                   for t in range(RT)]
            msb = pers.tile([128, RT], F32, name="msb", tag="msb")
            xrTp = pers.tile([128, 2, r], F8, name="xrTp", tag="xrTp")

            nc.gpsimd.dma_start(wkt[:], wk_d[:])
            nc.gpsimd.dma_start(wvt[:], wv_d[:])
            nc.gpsimd.dma_start(wqt[:], wq_d[:])
            nc.gpsimd.dma_start(xrTp[:], xrTp_d[:])
            nc.gpsimd.dma_start(bk2[:], bk_d[:])
            nc.gpsimd.dma_start(bq2[:], bq_d[:])
            nc.vector.memset(ones8[:], 1.0)
            epsb = pers.tile([128, 1], F32, name="epsb", tag="epsb")
            nc.vector.memset(epsb[:], 1e-24)

            def evac(eng, out_ap, in_ap, bias=None, scale=None, relu=False):
                """PSUM -> SBUF evacuation on ACT or DVE."""
                if eng is nc.scalar:
                    func = AF.Relu if relu else (AF.Identity if bias is not None
                                                 else AF.Copy)
                    kw = {}
                    if bias is not None:
                        kw["bias"] = bias
                    if scale is not None:
                        kw["scale"] = scale
                    nc.scalar.activation(out_ap, in_ap, func, **kw)
                else:
                    if relu:
                        if scale is not None:
                            eng.tensor_scalar(out=out_ap, in0=in_ap,
                                              scalar1=0.0, scalar2=scale,
                                              op0=ALU.max, op1=ALU.mult)
                        else:
                            eng.tensor_scalar(out=out_ap, in0=in_ap,
                                              scalar1=0.0, scalar2=None,
                                              op0=ALU.max)
                    elif bias is not None:
                        eng.tensor_scalar_add(out_ap, in_ap, bias)
                    elif scale is not None:
                        eng.tensor_scalar_mul(out_ap, in_ap, scale)
                    else:
                        eng.tensor_copy(out_ap, in_ap)

            xts = {}

            def fetch(ch, nsplit=1):
                if ch >= CH or ch in xts:
                    return
                xt = xtp.tile([128, 2, 1024], F8, name="xt", tag="xt")
                w = 1024 // nsplit
                for i in range(nsplit):
                    sl = slice(i * w, (i + 1) * w)
                    gl = slice(ch * 1024 + i * w, ch * 1024 + (i + 1) * w)
                    nc.sync.dma_start(xt[:, :, sl], xTp_d[:, :, gl])
                xts[ch] = xt

            def emit_chunk(ch, wp):
                """Stream one 1024-col slab of xTp; produce kTp, vp, cs."""
                xt = xts.pop(ch)
                fetch(ch + 1)
                crd = wp.tile([128, 8], F32, name="crd", tag="crd", bufs=2)
                for jh in range(2):
                    for db in range(2):
                        kps = wp.tile([128, 512], F32, name="kps", tag="w")
                        for m in range(2):
                            lsl = slice(jh * 512 + m * 256,
                                        jh * 512 + (m + 1) * 256)
                            nc.tensor.matmul(kps[:, m * 256:(m + 1) * 256],
                                             wkt[:, :, db * 128:(db + 1) * 128],
                                             xt[:, :, lsl],
                                             start=True, stop=True,
                                             perf_mode=PM.DoubleRow)
                        gsl = slice(ch * 1024 + jh * 512,
                                    ch * 1024 + (jh + 1) * 512)
                        evac(kv_eng(), kTp[:, db, gsl], kps[:],
                             bias=bk2[:, db:db + 1])
                    # ksq from the fp8 kT slab (SBUF), column norms via PE
                    gsl = slice(ch * 1024 + jh * 512, ch * 1024 + (jh + 1) * 512)
                    ksq = ksqp.tile([128, 2, 512], F8, name="ksq", tag="ksq")
                    ke = ksq_eng()
                    ke.tensor_tensor(out=ksq[:], in0=kTp[:, :, gsl],
                                     in1=kTp[:, :, gsl], op=ALU.mult)
                    for t in range(4):
                        col = jh * 4 + t
                        nc.tensor.matmul(crd[:, col:col + 1],
                                         ksq[:, :, t * 128:(t + 1) * 128],
                                         ones8[:],
                                         start=True, stop=True,
                                         perf_mode=PM.DoubleRow)
                    # cs for this half chunk (shortens the v-evac chain)
                    hs = slice(ch * 8 + jh * 4, ch * 8 + (jh + 1) * 4)
                    cl = slice(jh * 4, (jh + 1) * 4)
                    csn = ep.tile([128, 4], F32, name="csn", tag="csn")
                    nc.scalar.activation(csn[:], crd[:, cl], AF.Sqrt,
                                         bias=epsb[:])
                    nc.vector.reciprocal(cs[:, hs], csn[:])
                    nc.gpsimd.tensor_copy(cs8[:, hs, 0:1], cs[:, hs])
                # v projection: [j, d] psums, 2 j-blocks per bank;
                # pure evac, then per-jb cs scaling in SBUF (Pool-friendly)
                for pj in range(4):
                    jj = ch * 4 + pj
                    vps = wp.tile([128, 512], F32, name="vps", tag="w")
                    for i in range(2):
                        lsl = slice((pj * 2 + i) * 128, (pj * 2 + i + 1) * 128)
                        nc.tensor.matmul(vps[:, i * 256:(i + 1) * 256],
                                         xt[:, :, lsl], wvt[:],
                                         start=True, stop=True,
                                         perf_mode=PM.DoubleRow)
                    evac(kv_eng(), vp[jj][:], vps[:])
                    for i in range(2):
                        jb = 2 * jj + i
                        se = vs_eng()
                        if se is nc.scalar:
                            nc.scalar.activation(vp[jj][:, i, :],
                                                 vp[jj][:, i, :], AF.Copy,
                                                 scale=cs[:, jb:jb + 1])
                        else:
                            se.tensor_scalar(out=vp[jj][:, i, :],
                                             in0=vp[jj][:, i, :],
                                             scalar1=cs[:, jb:jb + 1],
                                             scalar2=None, op0=ALU.mult)

            def emit_rproj(wp):
                """q / k_self / v_self projections for this core's rows."""
                for db in range(2):
                    for rh in range(r // 512):
                        rsl = slice(rh * 512, (rh + 1) * 512)
                        qps = wp.tile([128, 512], F32, name="qps", tag="w")
                        for m in range(2):
                            msl = slice(rh * 512 + m * 256,
                                        rh * 512 + (m + 1) * 256)
                            nc.tensor.matmul(qps[:, m * 256:(m + 1) * 256],
                                             wqt[:, :, db * 128:(db + 1) * 128],
                                             xrTp[:, :, msl],
                                             start=True, stop=True,
                                             perf_mode=PM.DoubleRow)
                        evac(kv_eng(), qTp[:, db, rsl], qps[:],
                             bias=bq2[:, db:db + 1])
                        kps = wp.tile([128, 512], F32, name="ksps", tag="w")
                        for m in range(2):
                            msl = slice(rh * 512 + m * 256,
                                        rh * 512 + (m + 1) * 256)
                            nc.tensor.matmul(kps[:, m * 256:(m + 1) * 256],
                                             wkt[:, :, db * 128:(db + 1) * 128],
                                             xrTp[:, :, msl],
                                             start=True, stop=True,
                                             perf_mode=PM.DoubleRow)
                        evac(kv_eng(), ksf[:, db, rsl], kps[:],
                             bias=bk2[:, db:db + 1])
                for tt in range(RT // 2):
                    vsp = wp.tile([128, 512], F32, name="vsp", tag="w")
                    for i in range(2):
                        t = 2 * tt + i
                        nc.tensor.matmul(vsp[:, i * 256:(i + 1) * 256],
                                         xrTp[:, :, t * 128:(t + 1) * 128],
                                         wvt[:],
                                         start=True, stop=True,
                                         perf_mode=PM.DoubleRow)
                    evac(kv_eng(), vself[tt][:], vsp[:])

            def emit_selfterm(wp):
                """m = relu(diag(q.k_self)) / |k_self| for diagonal removal."""
                for h in range(2):
                    hsl = slice(h * 512, (h + 1) * 512)
                    nc.gpsimd.tensor_tensor(out=qk8[:, :, hsl],
                                            in0=qTp[:, :, hsl],
                                            in1=ksf[:, :, hsl], op=ALU.mult)
                    if QS_PAT[h % len(QS_PAT)] == "A":
                        nc.scalar.activation(qs8[:, :, hsl], ksf[:, :, hsl],
                                             AF.Square)
                    else:
                        nc.gpsimd.tensor_tensor(out=qs8[:, :, hsl],
                                                in0=ksf[:, :, hsl],
                                                in1=ksf[:, :, hsl],
                                                op=ALU.mult)
                sdkp = wp.tile([128, 2, RT], F32, name="sdkp", tag="crd",
                               bufs=2)
                for t in range(RT):
                    tsl = slice(t * 128, (t + 1) * 128)
                    nc.tensor.matmul(sdkp[:, 0, t:t + 1], qk8[:, :, tsl],
                                     ones8[:], start=True, stop=True,
                                     perf_mode=PM.DoubleRow)
                    nc.tensor.matmul(sdkp[:, 1, t:t + 1], qs8[:, :, tsl],
                                     ones8[:], start=True, stop=True,
                                     perf_mode=PM.DoubleRow)
                kst = ep.tile([128, RT], F32, name="kst", tag="kst")
                nc.scalar.activation(kst[:], sdkp[:, 1, :], AF.Sqrt,
                                     bias=epsb[:])
                inv = ep.tile([128, RT], F32, name="inv", tag="inv")
                nc.vector.reciprocal(inv[:], kst[:])
                nc.vector.tensor_scalar(out=msb[:], in0=sdkp[:, 0, :],
                                        scalar1=0.0, scalar2=None, op0=ALU.max)
                nc.gpsimd.tensor_tensor(out=msb[:], in0=msb[:], in1=inv[:],
                                        op=ALU.mult)

            def emit_scores(rb, g, sp, avm, avo):
                """4 j-blocks of scores -> relu -> fp8 wt4 -> AV accumulate."""
                rsl = slice(rb * RW, (rb + 1) * RW)
                sc = sp.tile([128, 1024], F32, name="sc", tag="sc")
                for i in range(GJB):
                    jb = g * GJB + i
                    nc.tensor.matmul(sc[:, i * 256:(i + 1) * 256],
                                     kTp[:, :, jb * 128:(jb + 1) * 128],
                                     qTp[:, :, rsl],
                                     start=True, stop=True,
                                     perf_mode=PM.DoubleRow)
                wt4 = wtp.tile([128, GJB, 256], F8, name="wt4", tag="wt4")
                evac(sc_eng(), wt4[:], sc[:], relu=True)
                for pj in range(GJB // 2):
                    jj = g * 2 + pj
                    for s in range(SS):
                        ssl = slice(s * 128, (s + 1) * 128)
                        nc.tensor.matmul(avm[:, s, :],
                                         wt4[:, 2 * pj:2 * pj + 2, ssl],
                                         vp[jj][:],
                                         start=(jj == 0),
                                         stop=(jj == NJ // 2 - 1),
                                         perf_mode=PM.DoubleRow)
                        nc.tensor.matmul(avo[:, s:s + 1],
                                         wt4[:, 2 * pj:2 * pj + 2, ssl],
                                         cs8[:, 2 * jj:2 * jj + 2, :],
                                         start=(jj == 0),
                                         stop=(jj == NJ // 2 - 1),
                                         perf_mode=PM.DoubleRow)

            def emit_epilogue(rb, avm, avo):
                avv = ep.tile([128, SS, 256], F32, name="avv", tag="avv")
                nc.scalar.activation(avv[:], avm[:], AF.Copy)
                dens = ep.tile([128, SS], F32, name="dens", tag="dens")
                nc.vector.tensor_scalar_add(dens[:], avo[:], 1e-9)
                for s in range(SS):
                    t = rb * SS + s
                    tmp = ep.tile([128, D], F32, name="tmp", tag="tmp")
                    nc.gpsimd.tensor_scalar(out=tmp[:],
                                            in0=vself[t // 2][:, t % 2, :],
                                            scalar1=msb[:, t:t + 1],
                                            scalar2=None, op0=ALU.mult)
                    num2 = ep.tile([128, D], F32, name="num2", tag="num2")
                    nc.gpsimd.tensor_tensor(out=num2[:], in0=avv[:, s, :],
                                            in1=tmp[:], op=ALU.subtract)
                    den = ep.tile([128, 1], F32, name="den", tag="den")
                    nc.vector.tensor_scalar(out=den[:], in0=dens[:, s:s + 1],
                                            scalar1=msb[:, t:t + 1],
                                            scalar2=None, op0=ALU.subtract)
                    rec = ep.tile([128, 1], F32, name="rec", tag="rec")
                    nc.vector.reciprocal(rec[:], den[:])
                    ot1 = ep.tile([128, D], F32, name="ot1", tag="ot1")
                    nc.gpsimd.tensor_scalar(out=ot1[:], in0=num2[:],
                                            scalar1=rec[:], scalar2=None,
                                            op0=ALU.mult)
                    ot = otp.tile([128, D], F32, name="ot", tag="ot")
                    nc.gpsimd.tensor_tensor(out=ot[:], in0=ot1[:],
                                            in1=xrt[t][:], op=ALU.add)
                    nc.sync.dma_start(out_d[t * 128:(t + 1) * 128, :], ot[:])

            # ---- phase A: all projections (PSUM: 2 k/q banks + 2x2 v
            # banks + 2 colsum banks) ----
            with tc.tile_pool(name="wp", bufs=6, space="PSUM") as wp:
                fetch(0, nsplit=4)
                emit_chunk(0, wp)
                emit_rproj(wp)
                emit_selfterm(wp)
                for t in range(RT):
                    nc.gpsimd.dma_start(xrt[t][:],
                                        xr_d[t * 128:(t + 1) * 128, :])
                for ch in range(1, CH):
                    emit_chunk(ch, wp)

            # ---- phase B: scores + w@v (3 x 2-bank scores + 2 accum) ----
            with tc.tile_pool(name="sp", bufs=3, space="PSUM") as sp, \
                 tc.tile_pool(name="avp", bufs=1, space="PSUM") as avp:
                for rb in range(NRB):
                    avm = avp.tile([128, SS, 256], F32, name="avm", tag="avm")
                    avo = avp.tile([128, SS], F32, name="avo", tag="avo")
                    for g in range(NG):
                        emit_scores(rb, g, sp, avm, avo)
                    emit_epilogue(rb, avm, avo)
    nc.compile()
    return nc


def _get_nc(n=N, r=N // M):
    key = (n, r)
    if key not in _CACHE:
        _CACHE[key] = build(n, r)
    return _CACHE[key]


def _pairT(a2d):
    """[n, 256] -> fp8 pair layout [128, 2, n] (transposed)."""
    f8 = mybir.dt.np(F8)
    a = np.asarray(a2d, np.float32).astype(f8)
    n = a.shape[0]
    return np.ascontiguousarray(a.T.reshape(2, 128, n).transpose(1, 0, 2))


def kernel(x, Wq, bq, Wk, bk, Wv, bv):
    global LAST
    x = np.ascontiguousarray(np.asarray(x, np.float32))
    n = x.shape[0]
    r = n // M
    xTp = _pairT(x)
    wqTp = _pairT(np.asarray(Wq, np.float32))   # == Wq.T in pair layout
    wkTp = _pairT(np.asarray(Wk, np.float32))
    wvTp = _pairT(np.asarray(Wv, np.float32))
    xplus = x + np.asarray(bv, np.float32)[None, :]
    bq2 = np.ascontiguousarray(np.asarray(bq, np.float32).reshape(2, 128).T)
    bk2 = np.ascontiguousarray(np.asarray(bk, np.float32).reshape(2, 128).T)
    in_maps = []
    for c in range(M):
        rows = slice(c * r, (c + 1) * r)
        in_maps.append({
            "xTp": xTp,
            "xrTp": _pairT(x[rows]),
            "xr": np.ascontiguousarray(xplus[rows]),
            "wqTp": wqTp, "wkTp": wkTp, "wvTp": wvTp,
            "bq2": bq2, "bk2": bk2,
        })
    res = run_bass_kernel_spmd(_get_nc(n, r), in_maps, core_ids=list(range(M)),
                               trace=TRACE)
    LAST = res
    return np.concatenate([res.results[c]["out"] for c in range(M)], axis=0)
